# revision 67
# baseline (speedup 1.0000x reference)
"""Self-contained TRN2 kernel for nn_Block_41695542510261 (dense transformer block).

Accepts FULL unsharded inputs, distributes across 8 NeuronCores internally
(2 cores per batch element, causal-balanced 64-row query chunks), returns
the FULL [4, 1024, 1024] output. See build_nc docstring for the design.
"""
import sys, os
for _p in ('/opt/trn_rl_repo', '/root/.axon_site/_ro/trn_rl_repo'):
    if os.path.isdir(_p) and _p not in sys.path:
        sys.path.insert(0, _p)
"""Transformer block kernel for TRN2 — 8-core SPMD, feature-major layout.

Reference: pre-LN attention block + SwiGLU FFN, B=4 T=1024 C=1024 H=16 D=64 DFF=4096.

Sharding: core c handles batch b=c//2, parity par=c%2. Each batch's 16
64-row query chunks split by parity: position p=0..7 <-> chunk j=2p+par.
Causal key-tile count for position p is p+1 for BOTH parities, so one
uniform SPMD program serves all 8 cores. Odd cores receive x with
adjacent 64-column blocks swapped so "own" tokens always sit at even
block positions (compile-time APs stay uniform); key order inside each
128-key tile is permuted consistently for K/V/mask, which attention
sums are invariant to.

Layout: all activations feature-major (xT[c, t]). LN stats via
ones-matmul over the partition (channel) dim + PE outer-product
broadcast. Attention computes S^T = (q.k)^T directly (lhsT=kT, rhs=qT),
softmax without max subtraction (scores bounded; scale 1/32 applied in
the exp), causal masking via 0/1 multiply on the single diagonal key
tile, denominator via a ones-column appended to V, normalization via a
K=1 outer-product matmul (hi/lo split for near-fp32 precision).

Matmuls run in bf16 with fp32 PSUM accumulation, except six fp8
(e4m3, DoubleRow = 2x PE rate) conversions chosen via a numpy
quantization sim validated against measured hw error (sim tracks hw
within ~0.1e-2): Wq/Wk projections, the V projection (its output is
fp8 for AV anyway, so ~free), the FFN val path (w2), gv@w3, P/V in
the attention AV matmul (kt-pair DoubleRow), and half the w1 gate
contraction (full-fp8 gate and fp8 Wo both measured over the 2e-2
gate). Weights are host-scaled by powers of 2 to clear fp8
subnormals; scales divide out in the exp scale / activation scale /
output scale. Weights are also host-repacked partition-major so DMA
partition lines are 1-2KB contiguous (128B lines ran the DMA engine
at ~40GB/s and stalled the w3 phase). Wo's first ci-half runs during
late attention as real filler. The residual path stays fp32.

Reciprocals (LN rsqrt and the softmax denominator) run on the scalar
engine as exp(-ln(x)) / exp(-0.5 ln(x)) (~5e-5 rel, measured): the
DVE RECIPROCAL (1.8-3.3us, free-dim-serial) had been the dominant
PE-stall edge via the score-tile PSUM ring. Ln/Exp/Square share one
act table set; the Silu set is preloaded via a dummy activation
data-pinned after LN2's Exp (a no-dep activation gets hoisted by the
scheduler and forces two extra table swaps).

LN chains write their quantized consumers directly (fast8 path):
bf16-quality mu/rs broadcasts suffice because every consumer is fp8
or bf16-quantized, and the 2-byte vector normalize chain halves the
serial cost. The stats ones-vector carries 1/C so stats matmuls
produce mu / E[x^2] directly.

keep_warm matmuls hold the HAM activity clock at k=8 (k=4 halves the
PE clock; idle quanta trigger it). Warm blocks that should fill a
specific stall are data-pinned (keep_warm_on) to a just-produced
tile; the scheduler hoists dependency-free matmuls away from their
emission point. 256-col warms double LDWEIGHTS overhead (~+27us) —
keep 512-col in hot paths.
"""
import contextlib
import json
import numpy as np
import ml_dtypes

import concourse.bass as bass
import concourse.mybir as mybir
import concourse.tile as tile

f32 = mybir.dt.float32
bf16 = mybir.dt.bfloat16
f8 = mybir.dt.float8e4
AF = mybir.ActivationFunctionType
DR = mybir.MatmulPerfMode.DoubleRow

C = 1024        # d_model
T = 1024        # seq len
H = 16          # heads
D = 64          # head dim
DFF = 4096
TOK = 512       # own tokens per core
NCT = C // 128  # 8 c tiles
NTT = T // 128  # 8 token (key) tiles
NP = H // 2     # 8 head pairs
NCH = 8         # q chunks per core (64 rows each)
EPS = 1e-5

# fp8 (e4m3, DoubleRow 2x matmul) coverage. Error budget per numpy sim:
# qk is ~free (softmax washes it out), val + gvw3 together land ~1.5e-2
# max-rel vs the 2e-2 gate. v/gate/ctxwo stay bf16 (worst err/perf ratio).
FP8_QK = True    # Wq/Wk projections (h, weights fp8)
FP8_VAL = True   # FFN w2 (val) path
FP8_GVW3 = True  # gv -> w3 matmul
FP8_AV = True    # P (exp out), v_all, mask in fp8; AV via DoubleRow kt pairs
FP8_GATE4 = True # FFN w1 (gate): first 4 of 8 ct tiles fp8-DR, rest bf16
FP8_GATE_FULL = False  # FFN w1 fully fp8-DR (hw: 2.096e-2, over gate)
FP8_V = True     # V projection fp8-DR (v_all is fp8 anyway: ~free error)
LN_FAST = True   # bf16 mu/rs LN chains writing fp8/bf16 dst directly
DEN_BUFS = 0     # >0: pden gets its own PSUM pool (sps shrinks to 2)
S1 = 256.0       # host scale on w1 (fp8: clears subnormals; |w1*S1|max ~12 << 448)
S_QK = 64.0      # host scale on Wq/Wk (w~0.02 must clear fp8 subnormals)
S_V = 64.0       # host scale on Wv (fp8)
S2 = 32.0        # host scale on w2 (keeps |gv*S2| < 240; 128 overflows)
S3 = 256.0       # host scale on w3 (fp8 subnormal clearance)
PO_SCALE = 1.0 / (S2 * S3)
SM_SCALE = (1.0 / 32.0) / (S_QK * S_QK if FP8_QK else 1.0)  # 1/sqrt(d_model)


def split_multiwaits(bir_bytes: bytes) -> bytes:
    """Split multi-wait instructions into single-wait EventSemaphore
    carriers placed just before them on the same engine. This walrus
    build has one sync-wait slot for several ISA structs (self-loading
    matmuls, direct DMAs, drains)."""
    m = json.loads(bir_bytes)
    ctr = 0
    for f in m['functions']:
        for blk in f.get('blocks', []):
            insts = blk.get('instructions', [])
            out = []
            changed = False
            for i in insts:
                si = i.get('sync_info')
                w = (si or {}).get('on_wait') or []
                if len(w) > 1:
                    for extra in w[:-1]:
                        ctr += 1
                        out.append({
                            'debug': i.get('debug'),
                            'engine': i['engine'],
                            'ins': [], 'outs': [],
                            'name': f'I-esw-{ctr}',
                            'opcode': 'EventSemaphore',
                            'sync_info': {'on_update': [], 'on_wait': [extra]},
                        })
                    si['on_wait'] = [w[-1]]
                    changed = True
                out.append(i)
            if changed:
                blk['instructions'] = out
    return json.dumps(m).encode()


def patch_nc(nc):
    orig = nc.to_json_bytes
    nc.to_json_bytes = lambda: split_multiwaits(orig())
    return nc


def build_nc(causal=True, sz=None, silu_act=True):
    sz = sz or {}
    nc = bass.Bass(trn_type="TRN2", target_bir_lowering=False, debug=False)

    xt = nc.dram_tensor("xt", [C, T], f32, kind="ExternalInput")
    av_dt = f8 if FP8_AV else bf16
    if causal:
        maskt = nc.dram_tensor("maskt", [128, 2, 128], av_dt, kind="ExternalInput")
    else:
        maskt = nc.dram_tensor("maskt", [4, NTT, 128, 128], av_dt, kind="ExternalInput")
    qk_dt = f8 if FP8_QK else bf16
    w2_dt = f8 if FP8_VAL else bf16
    w3_dt = f8 if FP8_GVW3 else bf16
    # weights arrive pre-transposed to partition-major [.., 128p, ..] so each
    # DMA partition line is 1-2KB contiguous (128B lines ran the DMA engine
    # at ~40GB/s and stalled the w3 phase)
    wq = nc.dram_tensor("wq", [NP, 128, NCT, 128], qk_dt, kind="ExternalInput")
    wk = nc.dram_tensor("wk", [NP, 128, NCT, 128], qk_dt, kind="ExternalInput")
    wv = nc.dram_tensor("wv", [4, 128, NCT, 256], f8 if FP8_V else bf16,
                        kind="ExternalInput")
    wo = nc.dram_tensor("wo", [NCT, 2, 128, 4, 128], bf16, kind="ExternalInput")
    if FP8_GATE_FULL:
        w1f = nc.dram_tensor("w1f", [32, 128, NCT, 128], f8, kind="ExternalInput")
        w1a = w1b = None
    else:
        w1a_dt = f8 if FP8_GATE4 else bf16
        w1a = nc.dram_tensor("w1a", [32, 128, 4, 128], w1a_dt, kind="ExternalInput")
        w1b = nc.dram_tensor("w1b", [32, 128, 4, 128], bf16, kind="ExternalInput")
    w2 = nc.dram_tensor("w2", [32, 128, NCT, 128], w2_dt, kind="ExternalInput")
    w3 = nc.dram_tensor("w3", [NCT, 4, 128, 8, 128], w3_dt, kind="ExternalInput")
    # packed per-channel constants: one DMA instead of eight (each
    # DMA_DIRECT2D costs ~650ns of sync-queue issue time at startup, which
    # delayed the critical xT input fetch)
    cvec = nc.dram_tensor("cvec", [128, 6, NCT], f32, kind="ExternalInput")
    bvec = nc.dram_tensor("bvec", [128, 2, 32], f32, kind="ExternalInput")
    ident = nc.dram_tensor("ident", [128, 128], bf16, kind="ExternalInput")
    out = nc.dram_tensor("out", [C, TOK], f32, kind="ExternalOutput")

    def cnt(p):
        return (p + 1) if causal else NTT

    with tile.TileContext(nc) as tc, contextlib.ExitStack() as ctx:
        consts = ctx.enter_context(tc.tile_pool(name="consts", bufs=1))
        perB = ctx.enter_context(tc.tile_pool(name="perB", bufs=1))
        w12p = ctx.enter_context(tc.tile_pool(name="w12", bufs=6))
        w3p = ctx.enter_context(tc.tile_pool(name="w3_sb", bufs=8))

        # ---- constants ----
        ones_row = consts.tile([1, 128], bf16)
        nc.vector.memset(ones_row, 1.0)
        ones_col = consts.tile([128, 1], bf16)
        # carries the 1/C stats normalization (2^-10, exact in bf16): the
        # stats matmuls then produce mu / E[x^2] directly, removing two
        # serial scalar muls from every LN finish chain
        nc.vector.memset(ones_col, 1.0 / C)
        eps_t = consts.tile([1, 1], f32)
        nc.vector.memset(eps_t, EPS)
        warm_t = consts.tile([128, 512], bf16)
        nc.vector.memset(warm_t, 0.0)
        warm_t8 = consts.tile([128, 128], f8)
        nc.vector.memset(warm_t8, 0.0)
        warm_tf = consts.tile([128, 128], f32)
        nc.vector.memset(warm_tf, 0.0)

        def keep_warm(pool, n, cols=512):
            # dependency-free matmuls that execute during upcoming PE
            # dependency stalls, keeping the HAM clock gate up; smaller
            # cols = finer granularity = less real-work delay
            wp = pool.tile([128, 512], f32, tag="wp")
            for _ in range(n):
                nc.tensor.matmul(wp[:, 0:cols], lhsT=warm_t[:, 0:128],
                                 rhs=warm_t[:, 0:cols], start=True, stop=True)

        def keep_warm_on(pool, n, rhs):
            # dep-pinned warm: the scheduler hoists dependency-free matmuls
            # away from the stall they're meant to fill; reading a
            # just-produced tile anchors them at the right spot
            cols = rhs.free_size()
            wp = pool.tile([128, 512], f32, tag="wp")
            # f32 rhs is NOT supported here: an f32 warm matmul corrupted
            # downstream fp8 results on this hw (PE mode interaction)
            assert rhs.dtype != f32
            lhs = warm_t8 if rhs.dtype == f8 else warm_t
            for _ in range(n):
                nc.tensor.matmul(wp[:, 0:cols], lhsT=lhs[:, 0:128],
                                 rhs=rhs, start=True, stop=True)

        # ---- phase-B persistent tiles (live to the end) ----
        x2T = perB.tile([128, NCT, TOK], f32)
        ln2_fast = LN_FAST and sz.get('ln2', False)
        x2b = None
        if ln2_fast:
            # bf16 cast of x2 (stats side-product) feeds the 2-byte LN2 chain
            x2b = perB.tile([128, NCT, TOK], bf16, tag="x2b")
        h2T = None
        if not FP8_GATE_FULL:
            h2T = perB.tile([128, NCT, TOK], bf16, tag="h2T")
        if FP8_VAL or FP8_GATE4 or FP8_GATE_FULL:
            h2q = perB.tile([128, NCT, TOK], f8)
        else:
            h2q = h2T
        outT = perB.tile([128, NCT, TOK], f32)

        # own-token columns (even 64-blocks) of [:, ct, :]
        def own(tl, ct):
            return tl[:, ct, :].rearrange(
                "p (j two i) -> p j two i", two=2, i=64)[:, :, 0, :]

        # ---------- feature-major layer norm ----------
        def ln_begin(stps, ntok):
            mean_ps = stps.tile([1, ntok], f32, tag="mean")
            ex2_ps = stps.tile([1, ntok], f32, tag="ex2")
            return mean_ps, ex2_ps

        def ln_stats_prep(src_ap, ntok, sqp, xb_dst=None):
            if xb_dst is None:
                xb = sqp.tile([128, ntok], bf16, tag="xb")
            else:
                xb = xb_dst
            nc.vector.tensor_copy(out=xb, in_=src_ap)
            sq = sqp.tile([128, ntok], bf16, tag="sq")
            # square on the scalar engine: vector is the busier queue here
            nc.scalar.activation(sq, xb, AF.Square)
            return xb, sq

        def ln_stats_mm(stats, prep, ct):
            # emitted a couple of cts behind the prep so the in-order PE
            # stream never waits on the vector/scalar prep chain
            mean_ps, ex2_ps = stats
            xb, sq = prep
            nc.tensor.matmul(mean_ps, lhsT=ones_col, rhs=xb,
                             start=(ct == 0), stop=(ct == NCT - 1))
            nc.tensor.matmul(ex2_ps, lhsT=ones_col, rhs=sq,
                             start=(ct == 0), stop=(ct == NCT - 1))

        # src(ct) -> [128, ntok] f32; writes dst(ct) (bf16) or fast8 dst8
        def layer_norm_T(src, dst, ntok, gs, bes, skip_affine,
                         sqp, stps, stss, bcp, tmpp, fast8=None):
            nh = ntok // 512
            assert nh == 1
            stats = ln_begin(stps, ntok)
            preps = []
            for ct in range(NCT):
                xbd = fast8['xb'](ct) if fast8 else None
                preps.append(ln_stats_prep(src(ct), ntok, sqp, xb_dst=xbd))
                if len(preps) >= 3:
                    ct_mm = ct - 2
                    ln_stats_mm(stats, preps[ct_mm], ct_mm)
            for ct_mm in (NCT - 2, NCT - 1):
                ln_stats_mm(stats, preps[ct_mm], ct_mm)
            ln_finish(stats, src, dst, ntok, gs, bes, skip_affine,
                      stss, bcp, tmpp, fast8=fast8)

        def ln_finish(stats, src, dst, ntok, gs, bes, skip_affine,
                      stss, bcp, tmpp, wp_pool=None, n_warm=24, fast8=None):
            mean_ps, ex2_ps = stats
            if fast8 is not None:
                keep_warm_on(wp_pool or bcp, n_warm, fast8['xb'](NCT - 2))
            else:
                keep_warm(wp_pool or bcp, n_warm)
            mu = mean_ps  # ones_col carries 1/C: PSUM rows are mu / E[x^2]
            musq = stss.tile([1, ntok], f32, tag="musq")
            nc.scalar.activation(musq, mean_ps, AF.Square)
            var = stss.tile([1, ntok], f32, tag="var")
            nc.vector.tensor_sub(var, ex2_ps, musq)
            # rs = exp(-0.5*ln(var+eps)) on the scalar engine: ~3e-5 rel
            # (measured), replaces Sqrt + the 3.3us single-partition DVE
            # reciprocal on the critical LN chain
            lnv = stss.tile([1, ntok], f32, tag="lnv")
            nc.scalar.activation(lnv, var, AF.Ln, bias=eps_t)
            rs = stss.tile([1, ntok], f32, tag="rs")
            nc.scalar.activation(rs, lnv, AF.Exp, scale=-0.5)
            mu_hi = stss.tile([1, ntok], bf16, tag="mu_hi")
            nc.vector.tensor_copy(out=mu_hi, in_=mu)
            if fast8 is None:
                rs_hi = stss.tile([1, ntok], bf16, tag="rs_hi")
                nc.vector.tensor_copy(out=rs_hi, in_=rs)
            if fast8 is not None:
                # consumers are fp8 (or bf16-quantized anyway): bf16-quality
                # mu/rs suffice, so use single-pass broadcasts and a 2-byte
                # vector normalize chain writing the quantized dst directly
                # (halves the serial chain and removes the cast trail that
                # gated the first downstream matmul)
                assert skip_affine
                mu_bc = bcp.tile([128, ntok], f32, tag="mu_bc")
                rs_bc = bcp.tile([128, ntok], f32, tag="rs_bc")
                nc.tensor.matmul(mu_bc, lhsT=ones_row, rhs=mu_hi,
                                 start=True, stop=True)
                # redundant mu-broadcasts bridge the PE over the Ln/Exp wait
                wpb = (wp_pool or bcp).tile([128, 512], f32, tag="wp")
                for _ in range(2):
                    nc.tensor.matmul(wpb[:, 0:ntok], lhsT=ones_row, rhs=mu_hi,
                                     start=True, stop=True)
                rs_hi = stss.tile([1, ntok], bf16, tag="rs_hi")
                nc.vector.tensor_copy(out=rs_hi, in_=rs)
                nc.tensor.matmul(rs_bc, lhsT=ones_row, rhs=rs_hi,
                                 start=True, stop=True)
                mu_sb = stss.tile([128, ntok], bf16, tag="mu_sb")
                nc.scalar.copy(out=mu_sb, in_=mu_bc)
                rs_sb = stss.tile([128, ntok], bf16, tag="rs_sb")
                nc.scalar.copy(out=rs_sb, in_=rs_bc)
                pw_n, pw_cols = fast8.get('post_warm', (12, 256))
                keep_warm_on(wp_pool or bcp, pw_n, mu_sb[:, 0:pw_cols])
                if fast8.get('preload_silu'):
                    # pull the Silu table swap off the first FFN activation:
                    # load it now, hidden behind the vector normalize chain.
                    # input dep on rs pins it AFTER the Ln/Exp pair (a no-dep
                    # activation gets hoisted by the scheduler and forces two
                    # extra table swaps)
                    dummy = stss.tile([1, 1], f32, tag="dummy")
                    nc.scalar.activation(dummy, rs[0:1, 0:1], AF.Silu)
                for ct in range(NCT):
                    # fp8-out mul costs 1.37us vs 0.41us bf16 (fast DVE mode
                    # lost), but skipping the bf16 intermediate is worth
                    # ~0.24e-2 of the error budget (double rounding)
                    tmpb = tmpp.tile([128, ntok], bf16, tag="lntmpb")
                    nc.vector.tensor_sub(tmpb, fast8['xb'](ct), mu_sb)
                    nc.vector.tensor_mul(fast8['dst8'](ct), tmpb, rs_sb)
                return
            # hi/lo split of mu and rs for near-fp32 broadcast
            mu_lob = stss.tile([1, ntok], bf16, tag="mu_lob")
            nc.vector.tensor_sub(mu_lob, mu, mu_hi)
            rs_lob = stss.tile([1, ntok], bf16, tag="rs_lob")
            nc.vector.tensor_sub(rs_lob, rs, rs_hi)
            mu_bc = bcp.tile([128, ntok], f32, tag="mu_bc")
            rs_bc = bcp.tile([128, ntok], f32, tag="rs_bc")
            nc.tensor.matmul(mu_bc, lhsT=ones_row, rhs=mu_hi,
                             start=True, stop=False)
            nc.tensor.matmul(mu_bc, lhsT=ones_row, rhs=mu_lob,
                             start=False, stop=True)
            nc.tensor.matmul(rs_bc, lhsT=ones_row, rhs=rs_hi,
                             start=True, stop=False)
            nc.tensor.matmul(rs_bc, lhsT=ones_row, rhs=rs_lob,
                             start=False, stop=True)
            for ct in range(NCT):
                tmp = tmpp.tile([128, ntok], f32, tag="lntmp")
                nc.vector.tensor_sub(tmp, src(ct), mu_bc)
                if skip_affine:
                    nc.vector.tensor_mul(dst(ct), tmp, rs_bc)
                else:
                    nc.vector.tensor_mul(tmp, tmp, rs_bc)
                    nc.scalar.activation(dst(ct), tmp, AF.Identity,
                                         bias=bes[:, ct:ct + 1],
                                         scale=gs[:, ct:ct + 1])

        with tc.tile_pool(name="perA", bufs=1) as perA:
            # ---- phase-A persistent tiles ----
            xT = perA.tile([128, NCT, T], f32)
            if LN_FAST and FP8_QK and FP8_V and sz.get('ln1', False):
                # h exists only as fp8; xbT (bf16 cast of x, stats
                # side-product) feeds the 2-byte normalize chain
                xbT = perA.tile([128, NCT, T], bf16)
                hT = None
                hq = perA.tile([128, NCT, T], f8)
            else:
                xbT = None
                hT = perA.tile([128, NCT, T], bf16)
                if FP8_QK:
                    hq = perA.tile([128, NCT, T], f8)
                else:
                    hq = hT
            v_all = perA.tile([128, NTT, H, 65], av_dt)
            ctxT = perA.tile([128, NCT, TOK], bf16)
            wo_all = perA.tile([128, NCT, 4, 128], bf16)
            x2h1 = perA.tile([128, NCT, TOK], bf16)

            for th in range(2):
                for ct in range(NCT):
                    nc.sync.dma_start(
                        out=xT[:, ct, th * 512:(th + 1) * 512],
                        in_=xt[ct * 128:(ct + 1) * 128, th * 512:(th + 1) * 512])
            # constants issued after the critical xT input stream
            cv = consts.tile([128, 6, NCT], f32)
            nc.sync.dma_start(out=cv, in_=cvec[:, :, :])
            g1s, be1s, g2s, be2s, bos, b3s = (cv[:, i, :] for i in range(6))
            bv = consts.tile([128, 2, 32], f32)
            nc.sync.dma_start(out=bv, in_=bvec[:, :, :])
            id_sb = consts.tile([128, 128], bf16)
            nc.sync.dma_start(out=id_sb, in_=ident[:, :])
            b1s = bv[:, 0, :]
            b2s = bv[:, 1, :]
            if causal:
                mk = consts.tile([128, 2, 128], av_dt)
                nc.sync.dma_start(out=mk, in_=maskt[:, :, :])
            else:
                mk = consts.tile([128, 4, NTT, 128], av_dt)
                nc.sync.dma_start(
                    out=mk, in_=maskt[:, :, :, :].rearrange("c k p q -> p c k q"))

            with tc.tile_pool(name="ln_sq", bufs=3) as sqp, \
                 tc.tile_pool(name="ln_st", bufs=1, space="PSUM") as stps, \
                 tc.tile_pool(name="ln_sts", bufs=1) as stss, \
                 tc.tile_pool(name="ln_bc", bufs=1, space="PSUM") as bcp, \
                 tc.tile_pool(name="ln_tmp", bufs=2) as tmpp, \
                 tc.tile_pool(name="wv_sb", bufs=1) as wvp, \
                 tc.tile_pool(name="v_ps", bufs=2, space="PSUM") as vps:
                # prefetch both V weight halves up front
                wv_dt = f8 if FP8_V else bf16
                wv_sbs = []
                for g in range(2):
                    wv_sb = wvp.tile([128, NCT, 2, 256], wv_dt, tag=f"wv{g}")
                    for q in range(2):
                        nc.sync.dma_start(
                            out=wv_sb[:, :, q, :], in_=wv[2 * g + q])
                    wv_sbs.append(wv_sb)

                def v_block(tts):
                    for g in range(2):
                        for tt in tts:
                            pv = vps.tile([128, 512], f32, tag="pv")
                            if FP8_V:
                                for g2 in range(4):
                                    nc.tensor.matmul(
                                        pv,
                                        lhsT=hq[:, 2 * g2:2 * g2 + 2,
                                                tt * 128:(tt + 1) * 128],
                                        rhs=wv_sbs[g][:, 2 * g2:2 * g2 + 2, :, :],
                                        perf_mode=DR,
                                        start=(g2 == 0), stop=(g2 == 3))
                                nc.scalar.activation(
                                    v_all[:, tt, 8 * g:8 * (g + 1), 0:64],
                                    pv[:, :].rearrange("p (h d) -> p h d", d=64),
                                    AF.Copy, scale=1.0 / S_V)
                            else:
                                for ct in range(NCT):
                                    nc.tensor.matmul(
                                        pv, lhsT=hT[:, ct, tt * 128:(tt + 1) * 128],
                                        rhs=wv_sbs[g][:, ct, :, :],
                                        start=(ct == 0), stop=(ct == NCT - 1))
                                nc.scalar.copy(
                                    out=v_all[:, tt, 8 * g:8 * (g + 1), 0:64],
                                    in_=pv[:, :].rearrange("p (h d) -> p h d", d=64))

                # LN half 0 -> V for its token tiles fills LN half 1's
                # dependency stall with real matmuls; then LN half 1 -> rest
                for th in range(2):
                    tsl = slice(th * 512, (th + 1) * 512)
                    if xbT is not None:
                        layer_norm_T(lambda ct: xT[:, ct, tsl], None, 512,
                                     g1s, be1s, sz.get('ln1', False),
                                     sqp, stps, stss, bcp, tmpp,
                                     fast8=dict(
                                         xb=lambda ct: xbT[:, ct, tsl],
                                         dst8=lambda ct: hq[:, ct, tsl],
                                         post_warm=(10, 256)))
                    else:
                        layer_norm_T(lambda ct: xT[:, ct, tsl],
                                     lambda ct: hT[:, ct, tsl], 512,
                                     g1s, be1s, sz.get('ln1', False),
                                     sqp, stps, stss, bcp, tmpp)
                    if FP8_QK and xbT is None and FP8_V:
                        # fp8 V consumes hq: cast before v_block
                        for ct in range(NCT):
                            nc.scalar.copy(out=hq[:, ct, tsl],
                                           in_=hT[:, ct, tsl])
                    v_block(range(4 * th, 4 * th + 4))
                    # after v_block so the casts don't block V evacuation
                    # on the in-order scalar queue
                    if FP8_QK and xbT is None and not FP8_V:
                        for ct in range(NCT):
                            nc.scalar.copy(out=hq[:, ct, tsl],
                                           in_=hT[:, ct, tsl])
            nc.vector.memset(v_all[:, :, :, 64:65], 1.0)
            for cot in range(NCT):
                nc.sync.dma_start(
                    out=wo_all[:, cot, :, :],
                    in_=wo[cot, 0])

            # ---------- attention ----------
            with tc.tile_pool(name="wqk", bufs=2) as wqkp, \
                 tc.tile_pool(name="qk_ps", bufs=1, space="PSUM") as qkps, \
                 tc.tile_pool(name="qk_sb", bufs=2) as qksb, \
                 tc.tile_pool(name="s_ps", bufs=(2 if DEN_BUFS else 3),
                              space="PSUM") as sps, \
                 tc.tile_pool(name="p_sb", bufs=2) as psb, \
                 tc.tile_pool(name="ctx_ps", bufs=3, space="PSUM") as cps, \
                 tc.tile_pool(name="at_wp", bufs=1, space="PSUM") as wps, \
                 tc.tile_pool(name="nrm_sb", bufs=2) as nsb, \
                 contextlib.ExitStack() as dctx:
                dps = dctx.enter_context(
                    tc.tile_pool(name="den_ps", bufs=DEN_BUFS, space="PSUM")) \
                    if DEN_BUFS else None
                def emit_scores_both(qT, kT, P0, P1, m):
                    # merged mp pair {2m, 2m+1}: one 256-wide q block per
                    # matmul — halves the small-matmul count (the attention
                    # phase is per-instruction-overhead bound, ~150ns fixed
                    # cost on a 107ns stream). h2=0 rows 0:64, h2=1 rows
                    # 64:128 co-execute via PE row packing.
                    n_kt = cnt(4 * m + 3)
                    qsl = slice(m * 256, (m + 1) * 256)
                    for kg in range(0, n_kt, 2):
                        # the block at kg==4m+2 is causally dead for sub-mp
                        # 2m (cols 0:128): compute/exp only the valid half
                        # and memset the dead half (sole writer: no cross-
                        # engine WAW race, no serialization)
                        dead = causal and kg == 4 * m + 2
                        csl = slice(128, 256) if dead else slice(0, 256)
                        qs2 = slice(qsl.start + csl.start, qsl.start + csl.stop)
                        ps0 = sps.tile([128, 2, 256], f32, tag="ps_s")
                        ps1 = sps.tile([128, 2, 256], f32, tag="ps_s")
                        for kt in range(kg, kg + 2):
                            nc.tensor.matmul(
                                ps0[:, kt - kg, csl],
                                lhsT=kT[0:64, kt * 128:(kt + 1) * 128],
                                rhs=qT[0:64, qs2], start=True, stop=True)
                            nc.tensor.matmul(
                                ps1[:, kt - kg, csl],
                                lhsT=kT[64:128, kt * 128:(kt + 1) * 128],
                                rhs=qT[64:128, qs2], start=True, stop=True)
                        nc.scalar.activation(P0[:, kg:kg + 2, csl],
                                             ps0[:, :, csl],
                                             AF.Exp, scale=SM_SCALE)
                        nc.scalar.activation(P1[:, kg:kg + 2, csl],
                                             ps1[:, :, csl],
                                             AF.Exp, scale=SM_SCALE)
                        if dead:
                            nc.vector.memset(P0[:, kg:kg + 2, 0:128], 0.0)
                            nc.vector.memset(P1[:, kg:kg + 2, 0:128], 0.0)
                    for P in (P0, P1):
                        if causal:
                            # diag masks: sub-mp 2m on cols 0:128 (kts
                            # 4m..4m+1), sub-mp 2m+1 on cols 128:256
                            nc.vector.tensor_mul(
                                P[:, 4 * m:4 * m + 2, 0:128],
                                P[:, 4 * m:4 * m + 2, 0:128], mk)
                            nc.vector.tensor_mul(
                                P[:, 4 * m + 2:4 * m + 4, 128:256],
                                P[:, 4 * m + 2:4 * m + 4, 128:256], mk)

                        else:
                            nc.vector.tensor_mul(P[:, 0:n_kt, 0:128],
                                                 P[:, 0:n_kt, 0:128],
                                                 mk[:, 2 * m, 0:n_kt, :])
                            nc.vector.tensor_mul(P[:, 0:n_kt, 128:256],
                                                 P[:, 0:n_kt, 128:256],
                                                 mk[:, 2 * m + 1, 0:n_kt, :])

                def emit_av(P, pctx, hp, m, h2):
                    n_kt = cnt(4 * m + 3)
                    h = 2 * hp + h2
                    if FP8_AV and n_kt % 2 == 0:
                        # DoubleRow over kt pairs: halves AV matmul count
                        npair = n_kt // 2
                        for j in range(npair):
                            nc.tensor.matmul(
                                pctx[:, h2, :],
                                lhsT=v_all[:, 2 * j:2 * j + 2, h, :],
                                rhs=P[:, 2 * j:2 * j + 2, :], perf_mode=DR,
                                start=(j == 0), stop=(j == npair - 1))
                    else:
                        for kt in range(n_kt):
                            nc.tensor.matmul(
                                pctx[:, h2, :], lhsT=v_all[:, kt, h, :],
                                rhs=P[:, kt, :],
                                start=(kt == 0), stop=(kt == n_kt - 1))

                def emit_den(pctx):
                    # hi/lo bf16 split of the softmax denominator row so the
                    # PE ones-broadcast reconstructs it at ~fp32 in PSUM
                    d_hi = nsb.tile([1, 512], bf16, tag="d_hi")
                    nc.vector.tensor_copy(out=d_hi, in_=pctx[64:65, :, :])
                    d_lo = nsb.tile([1, 512], bf16, tag="d_lo")
                    nc.vector.tensor_sub(d_lo, pctx[64:65, :, :], d_hi)
                    return d_hi, d_lo

                def emit_norm_pair(ga, gb):
                    # normalize two merged groups (= 4 original mp groups)
                    # with ONE 1/den chain: A on partitions 0:64, B on 64:128
                    if DEN_BUFS:
                        pden = dps.tile([128, 512], f32, tag="pden")
                    else:
                        pden = sps.tile([128, 512], f32, tag="ps_s")
                    for row, g in ((0, ga), (64, gb)):
                        if g is None:
                            continue
                        _, d_hi, d_lo, _, _ = g
                        nc.tensor.matmul(pden[row:row + 64, :],
                                         lhsT=ones_row[0:1, 0:64], rhs=d_hi,
                                         start=True, stop=False)
                        nc.tensor.matmul(pden[row:row + 64, :],
                                         lhsT=ones_row[0:1, 0:64], rhs=d_lo,
                                         start=False, stop=True)
                    # 1/den = exp(-ln(den)) on the scalar engine (~5e-5 rel,
                    # measured): frees the pden ring after the quick Ln and
                    # keeps the DVE RECIPROCAL off the busy vector queue
                    lnd = nsb.tile([128, 512], f32, tag="lnd")
                    nrm2 = nsb.tile([128, 512], f32, tag="nrm2")
                    if gb is None:
                        nc.scalar.activation(lnd[0:64, :], pden[0:64, :], AF.Ln)
                        nc.scalar.activation(nrm2[0:64, :], lnd[0:64, :],
                                             AF.Exp, scale=-1.0)
                    else:
                        nc.scalar.activation(lnd, pden, AF.Ln)
                        nc.scalar.activation(nrm2, lnd, AF.Exp, scale=-1.0)
                    for row, g in ((0, ga), (64, gb)):
                        if g is None:
                            continue
                        pctx, _, _, php, pm = g
                        for h2 in range(2):
                            nc.vector.tensor_mul(
                                ctxT[64 * h2:64 * (h2 + 1), php,
                                     pm * 256:(pm + 1) * 256],
                                pctx[0:64, h2, :],
                                nrm2[row:row + 64, 256 * h2:256 * (h2 + 1)])

                # pipeline over merged groups: scores(g+1) are emitted before
                # av(g) so the in-order PE stream never drains while the
                # exp/mask chain runs; norm/evac for g trails one group.
                # qk projection emission, split so pair hp+1's dense
                # N=512 matmuls can be interleaved into pair hp's attention
                # groups (fills exp/mask bubbles in the in-order PE stream)
                def own2(tl, g):
                    # own-token (even 64-block) columns for ct pair 2g, 2g+1
                    return tl[:, 2 * g:2 * g + 2, :].rearrange(
                        "p c (j two i) -> p c j two i", two=2, i=64)[:, :, :, 0, :]

                def make_qk_parts(hp):
                    wq_sb = wqkp.tile([128, NCT, 128], qk_dt, tag="wq")
                    nc.sync.dma_start(
                        out=wq_sb, in_=wq[hp])
                    wk_sb = wqkp.tile([128, NCT, 128], qk_dt, tag="wk")
                    nc.sync.dma_start(
                        out=wk_sb, in_=wk[hp])
                    qT = qksb.tile([128, 512], bf16, tag="qT")
                    kT = qksb.tile([128, 1024], bf16, tag="kT")

                    def part_q():
                        pq = qkps.tile([128, 512], f32, tag="pqk")
                        if FP8_QK:
                            for g in range(4):
                                nc.tensor.matmul(pq, lhsT=wq_sb[:, 2 * g:2 * g + 2, :],
                                                 rhs=own2(hq, g), perf_mode=DR,
                                                 start=(g == 0), stop=(g == 3))
                        else:
                            for ct in range(NCT):
                                nc.tensor.matmul(pq, lhsT=wq_sb[:, ct, :],
                                                 rhs=own(hT, ct),
                                                 start=(ct == 0), stop=(ct == NCT - 1))
                        nc.scalar.copy(out=qT, in_=pq)

                    def part_k(hh):
                        sl = slice(hh * 512, (hh + 1) * 512)
                        pk = qkps.tile([128, 512], f32, tag="pqk")
                        if FP8_QK:
                            for g in range(4):
                                nc.tensor.matmul(pk, lhsT=wk_sb[:, 2 * g:2 * g + 2, :],
                                                 rhs=hq[:, 2 * g:2 * g + 2, sl],
                                                 perf_mode=DR,
                                                 start=(g == 0), stop=(g == 3))
                        else:
                            for ct in range(NCT):
                                nc.tensor.matmul(pk, lhsT=wk_sb[:, ct, :],
                                                 rhs=hT[:, ct, sl],
                                                 start=(ct == 0), stop=(ct == NCT - 1))
                        nc.scalar.copy(out=kT[:, sl], in_=pk)

                    return qT, kT, (part_q, lambda: part_k(0), lambda: part_k(1))

                prev = None          # (P0, P1, hp, m) awaiting av
                pend = []            # groups awaiting a paired norm
                wo_early = list(range(NCT))  # Wo ci 0..3 half, run as filler
                qT, kT, parts = make_qk_parts(0)
                for pf in parts:
                    pf()
                nxt = None
                for hp in range(NP):
                    if hp + 1 < NP:
                        nxt = make_qk_parts(hp + 1)
                    for m in range(2):
                        P0 = psb.tile([128, NTT, 256], av_dt, tag="P0")
                        P1 = psb.tile([128, NTT, 256], av_dt, tag="P1")
                        emit_scores_both(qT, kT, P0, P1, m)
                        if hp >= 5 and wo_early:
                            cot = wo_early.pop(0)
                            pa1 = wps.tile([128, TOK], f32, tag="wp")
                            for ci in range(4):
                                nc.tensor.matmul(pa1, lhsT=wo_all[:, cot, ci, :],
                                                 rhs=ctxT[:, ci, :],
                                                 start=(ci == 0), stop=(ci == 3))
                            nc.scalar.copy(out=x2h1[:, cot, :], in_=pa1)
                            keep_warm_on(wps, 4, P0[:, 0:2, :])
                        else:
                            keep_warm(wps, 4)
                            keep_warm_on(wps, 3, P0[:, 0:2, :])
                        if hp + 1 < NP:
                            if m == 0:
                                nxt[2][0]()  # pair hp+1 q projection filler
                            else:
                                nxt[2][1]()  # pair hp+1 k halves
                                nxt[2][2]()
                        if len(pend) == 2:
                            emit_norm_pair(pend[0], pend[1])
                            pend = []
                        if prev is not None:
                            pP0, pP1, php, pm = prev
                            pctx = cps.tile([65, 2, 256], f32, tag="pctx")
                            emit_av(pP0, pctx, php, pm, 0)
                            emit_av(pP1, pctx, php, pm, 1)
                            d_hi, d_lo = emit_den(pctx)
                            pend.append((pctx, d_hi, d_lo, php, pm))
                        prev = (P0, P1, hp, m)
                    if hp + 1 < NP:
                        qT, kT, _ = nxt[0], nxt[1], None
                # drain
                if prev is not None:
                    pP0, pP1, php, pm = prev
                    pctx = cps.tile([65, 2, 256], f32, tag="pctx")
                    emit_av(pP0, pctx, php, pm, 0)
                    emit_av(pP1, pctx, php, pm, 1)
                    d_hi, d_lo = emit_den(pctx)
                    pend.append((pctx, d_hi, d_lo, php, pm))
                while pend:
                    ga = pend.pop(0)
                    gb = pend.pop(0) if pend else None
                    emit_norm_pair(ga, gb)
                    keep_warm(wps, 4)
                while wo_early:
                    cot = wo_early.pop(0)
                    pa1 = wps.tile([128, TOK], f32, tag="wp")
                    for ci in range(4):
                        nc.tensor.matmul(pa1, lhsT=wo_all[:, cot, ci, :],
                                         rhs=ctxT[:, ci, :],
                                         start=(ci == 0), stop=(ci == 3))
                    nc.scalar.copy(out=x2h1[:, cot, :], in_=pa1)

            # ---------- Wo + residual, LN2 stats interleaved per cot ----------
            with tc.tile_pool(name="wo_sb", bufs=2) as wop, \
                 tc.tile_pool(name="a_ps", bufs=2, space="PSUM") as aps, \
                 tc.tile_pool(name="a_sb", bufs=2) as asb, \
                 tc.tile_pool(name="l2_sq", bufs=3) as sqp2, \
                 tc.tile_pool(name="l2_st", bufs=1, space="PSUM") as stps2, \
                 tc.tile_pool(name="l2_sts", bufs=1) as stss2, \
                 tc.tile_pool(name="l2_bc", bufs=1, space="PSUM") as bcp2, \
                 tc.tile_pool(name="l2_tmp", bufs=2) as tmpp2:
                keep_warm(aps, 6)
                stats2 = ln_begin(stps2, TOK)
                preps2 = []
                for cot in range(NCT):
                    wo_sb = wop.tile([128, 4, 128], bf16, tag="wo")
                    nc.sync.dma_start(
                        out=wo_sb,
                        in_=wo[cot, 1])
                    pa = aps.tile([128, TOK], f32, tag="pa")
                    for ci in range(4):
                        nc.tensor.matmul(pa, lhsT=wo_sb[:, ci, :],
                                         rhs=ctxT[:, 4 + ci, :],
                                         start=(ci == 0), stop=False)
                    # fold the x2h1 (Wo first-half partial) add into the
                    # PSUM via identity weights: +0.43us on the starving PE
                    # here buys back a 1.37us f32 vector add per cot
                    nc.tensor.matmul(pa, lhsT=id_sb, rhs=x2h1[:, cot, :],
                                     start=False, stop=True)
                    if sz.get('bo', False):
                        nc.vector.tensor_add(x2T[:, cot, :], pa, own(xT, cot))
                    else:
                        tmpa = asb.tile([128, TOK], f32, tag="tmpa")
                        nc.scalar.activation(tmpa, pa, AF.Identity,
                                             bias=bos[:, cot:cot + 1], scale=1.0)
                        nc.vector.tensor_add(x2T[:, cot, :], tmpa, own(xT, cot))
                    preps2.append(ln_stats_prep(
                        x2T[:, cot, :], TOK, sqp2,
                        xb_dst=(x2b[:, cot, :] if ln2_fast else None)))
                    if len(preps2) >= 3:
                        ln_stats_mm(stats2, preps2[cot - 2], cot - 2)
                for ct_mm in (NCT - 2, NCT - 1):
                    ln_stats_mm(stats2, preps2[ct_mm], ct_mm)
                if ln2_fast:
                    n8 = NCT if FP8_GATE_FULL else 4

                    def ln2_dst8(ct):
                        return h2q[:, ct, :] if ct < n8 else h2T[:, ct, :]
                    ln_finish(stats2, lambda ct: x2T[:, ct, :], None, TOK,
                              g2s, be2s, True, stss2, bcp2, tmpp2,
                              wp_pool=aps, n_warm=10,
                              fast8=dict(xb=lambda ct: x2b[:, ct, :],
                                         dst8=ln2_dst8,
                                         post_warm=(12, 256),
                                         preload_silu=True))
                    if FP8_VAL and not FP8_GATE_FULL:
                        for ct in range(n8, NCT):
                            nc.scalar.copy(out=h2q[:, ct, :], in_=h2T[:, ct, :])
                else:
                    ln_finish(stats2, lambda ct: x2T[:, ct, :],
                              lambda ct: h2T[:, ct, :], TOK, g2s, be2s,
                              sz.get('ln2', False), stss2, bcp2, tmpp2,
                              wp_pool=aps, n_warm=10)
                    if FP8_VAL or FP8_GATE_FULL:
                        for ct in range(NCT):
                            nc.scalar.copy(out=h2q[:, ct, :], in_=h2T[:, ct, :])

        # ---------- FFN ----------
        with tc.tile_pool(name="g_ps", bufs=2, space="PSUM") as gps, \
             tc.tile_pool(name="vl_ps", bufs=2, space="PSUM") as vlps, \
             tc.tile_pool(name="g_sb", bufs=2) as gsbp, \
             tc.tile_pool(name="gv_sb", bufs=1) as gvp, \
             tc.tile_pool(name="o_ps", bufs=2, space="PSUM") as ops:
            gv_all = gvp.tile([128, 4, 8, TOK], w3_dt, tag="gv")
            # lookahead w3 weight fetch: issue DMAs well before the w3 loop
            # so its matmuls never wait on HBM
            w3_tiles = {}
            w3_next = [0]

            def w3_fetch_upto(n):
                while w3_next[0] < min(n, 4 * NCT):
                    i = w3_next[0]
                    cot, dc = divmod(i, 4)
                    t3 = w3p.tile([128, 8, 128], w3_dt, tag="w3")
                    nc.sync.dma_start(
                        out=t3,
                        in_=w3[cot, dc])
                    w3_tiles[i] = t3
                    w3_next[0] += 1

            for dc in range(4):
                gv = gv_all[:, dc, :, :]
                for fi in range(8):
                    ft = dc * 8 + fi
                    if dc == 3:
                        w3_fetch_upto(fi)
                    if FP8_GATE_FULL:
                        w1_sb = w12p.tile([128, NCT, 128], f8, tag="w1f")
                        nc.sync.dma_start(
                            out=w1_sb,
                            in_=w1f[ft])
                    else:
                        w1a_sb = w12p.tile([128, 4, 128], w1a_dt, tag="w1a")
                        nc.sync.dma_start(
                            out=w1a_sb, in_=w1a[ft])
                        w1b_sb = w12p.tile([128, 4, 128], bf16, tag="w1b")
                        nc.sync.dma_start(
                            out=w1b_sb, in_=w1b[ft])
                    w2_sb = w12p.tile([128, NCT, 128], w2_dt, tag="w2")
                    nc.sync.dma_start(
                        out=w2_sb, in_=w2[ft])
                    pg = gps.tile([128, TOK], f32, tag="pg")
                    pvl = vlps.tile([128, TOK], f32, tag="pvl")
                    if FP8_GATE_FULL:
                        for g in range(4):
                            nc.tensor.matmul(pg, lhsT=w1_sb[:, 2 * g:2 * g + 2, :],
                                             rhs=h2q[:, 2 * g:2 * g + 2, :],
                                             perf_mode=DR,
                                             start=(g == 0), stop=(g == 3))
                    elif FP8_GATE4:
                        for g in range(2):
                            nc.tensor.matmul(pg, lhsT=w1a_sb[:, 2 * g:2 * g + 2, :],
                                             rhs=h2q[:, 2 * g:2 * g + 2, :],
                                             perf_mode=DR,
                                             start=(g == 0), stop=False)
                    else:
                        for ci in range(4):
                            nc.tensor.matmul(pg, lhsT=w1a_sb[:, ci, :],
                                             rhs=h2T[:, ci, :],
                                             start=(ci == 0), stop=False)
                    if not FP8_GATE_FULL:
                        for ci in range(4):
                            nc.tensor.matmul(pg, lhsT=w1b_sb[:, ci, :],
                                             rhs=h2T[:, 4 + ci, :],
                                             start=False, stop=(ci == 3))
                    if FP8_VAL:
                        for g in range(4):
                            nc.tensor.matmul(pvl, lhsT=w2_sb[:, 2 * g:2 * g + 2, :],
                                             rhs=h2q[:, 2 * g:2 * g + 2, :],
                                             perf_mode=DR,
                                             start=(g == 0), stop=(g == 3))
                    else:
                        for ct in range(NCT):
                            nc.tensor.matmul(pvl, lhsT=w2_sb[:, ct, :],
                                             rhs=h2T[:, ct, :],
                                             start=(ct == 0), stop=(ct == NCT - 1))
                    gs_t = gsbp.tile([128, TOK], f32, tag="gs_t")
                    if silu_act:
                        nc.scalar.activation(gs_t, pg, AF.Silu,
                                             bias=b1s[:, ft:ft + 1], scale=1.0 / S1)
                    else:
                        # silu(x) = x * sigmoid(x); x = pg + b1
                        nc.scalar.activation(gs_t, pg, AF.Sigmoid,
                                             bias=b1s[:, ft:ft + 1], scale=1.0 / S1)
                        if sz.get('b1', False):
                            nc.vector.tensor_mul(gs_t, gs_t, pg)
                        else:
                            xg = gsbp.tile([128, TOK], f32, tag="xg")
                            nc.vector.tensor_scalar_add(xg, pg, b1s[:, ft:ft + 1])
                            nc.vector.tensor_mul(gs_t, gs_t, xg)
                    if sz.get('b2', False):
                        nc.vector.tensor_mul(gv[:, fi, :], pvl, gs_t)
                    else:
                        nc.vector.tensor_scalar_add(gv[:, fi, :], pvl,
                                                    b2s[:, ft:ft + 1])
                        nc.vector.tensor_mul(gv[:, fi, :], gv[:, fi, :], gs_t)
            # cot-major w3: all 4 dc chunks accumulate in one PSUM group,
            # one scale+add per output tile (replaces 32 vector accumulates)
            po_s = (S2 if FP8_VAL else 1.0) * (S3 if FP8_GVW3 else 1.0)
            for cot in range(NCT):
                po = ops.tile([128, TOK], f32, tag="po")
                for dc in range(4):
                    w3_fetch_upto(4 * cot + dc + 7)
                    w3_sb = w3_tiles.pop(4 * cot + dc)
                    if FP8_GVW3:
                        for g in range(4):
                            nc.tensor.matmul(po, lhsT=w3_sb[:, 2 * g:2 * g + 2, :],
                                             rhs=gv_all[:, dc, 2 * g:2 * g + 2, :],
                                             perf_mode=DR,
                                             start=(dc == 0 and g == 0),
                                             stop=(dc == 3 and g == 3))
                    else:
                        for fi in range(8):
                            nc.tensor.matmul(po, lhsT=w3_sb[:, fi, :],
                                             rhs=gv_all[:, dc, fi, :],
                                             start=(dc == 0 and fi == 0),
                                             stop=(dc == 3 and fi == 7))
                if not sz.get('b3', False):
                    tmpo = gsbp.tile([128, TOK], f32, tag="tmpo")
                    nc.scalar.activation(tmpo, po, AF.Identity,
                                         bias=b3s[:, cot:cot + 1],
                                         scale=1.0 / po_s)
                    nc.vector.tensor_add(outT[:, cot, :], tmpo, x2T[:, cot, :])
                else:
                    nc.vector.scalar_tensor_tensor(
                        out=outT[:, cot, :], in0=po, scalar=1.0 / po_s,
                        in1=x2T[:, cot, :], op0=mybir.AluOpType.mult,
                        op1=mybir.AluOpType.add)
                nc.sync.dma_start(out=out[cot * 128:(cot + 1) * 128, :],
                                  in_=outT[:, cot, :])
    patch_nc(nc)
    return nc


# ===================== host-side prep =====================

def swap_cols64(a):
    """swap adjacent 64-col blocks along last axis"""
    s = a.shape
    b = a.reshape(*s[:-1], s[-1] // 128, 2, 64)
    return b[..., ::-1, :].reshape(s)


def check_causal(mask):
    T_ = mask.shape[0]
    allow = ~np.isneginf(np.asarray(mask))
    allow_ref = ~np.triu(np.ones((T_, T_), bool), k=1)
    return np.array_equal(allow, allow_ref)


def make_mask_tiles(mask, causal):
    """per-core multiplicative mask tiles (bf16 0/1), key-order swapped for odd cores.

    Merged q-chunk pairs: positions {2mp, 2mp+1} share one N=128 block.
    Causal: one [128, 2, 128] tile — [:,0,:] masks key tile kt=2mp
    ([diag | ones]), [:,1,:] masks kt=2mp+1 ([zeros | diag]); the pattern
    is mp-independent. General: [4, 8, 128, 128] per (mp, kt)."""
    allow = ~np.isneginf(np.asarray(mask))  # [q, k] True = allowed
    tiles = []
    for core in range(8):
        par = core % 2

        def ktile_order(kt):
            k = np.arange(128 * kt, 128 * kt + 128)
            if par == 1:
                k = k.reshape(2, 64)[::-1].reshape(128)
            return k

        def qcols(mp):
            # merged block columns = positions 2mp, 2mp+1 -> chunks j=4mp+par, 4mp+2+par
            j0, j1 = 2 * (2 * mp) + par, 2 * (2 * mp + 1) + par
            return np.concatenate([np.arange(64 * j0, 64 * j0 + 64),
                                   np.arange(64 * j1, 64 * j1 + 64)])

        if causal:
            mp = 0
            m = np.zeros((128, 2, 128), dtype=(ml_dtypes.float8_e4m3 if FP8_AV else ml_dtypes.bfloat16))
            q = qcols(mp)
            for i, kt in enumerate((2 * mp, 2 * mp + 1)):
                m[:, i, :] = allow[np.ix_(q, ktile_order(kt))].T
            tiles.append(np.ascontiguousarray(m))
        else:
            m = np.zeros((4, NTT, 128, 128), dtype=(ml_dtypes.float8_e4m3 if FP8_AV else ml_dtypes.bfloat16))
            for mp in range(4):
                q = qcols(mp)
                for kt in range(NTT):
                    m[mp, kt] = allow[np.ix_(q, ktile_order(kt))].T
            tiles.append(m)
    return tiles


def prep_in_maps(inputs):
    bfl = ml_dtypes.bfloat16
    x = np.asarray(inputs['input'], np.float32)      # [B, T, C]
    mask = np.asarray(inputs['mask'], np.float32)
    causal = check_causal(mask)
    Wq = np.asarray(inputs['Wq'], np.float32)        # [H, C, D]
    Wk = np.asarray(inputs['Wk'], np.float32)
    Wv = np.asarray(inputs['Wv'], np.float32)
    Wo = np.asarray(inputs['Wo'], np.float32)        # [C, C]
    w1 = np.asarray(inputs['w1'], np.float32)        # [C, DFF]
    w2 = np.asarray(inputs['w2'], np.float32)
    w3 = np.asarray(inputs['w3'], np.float32)        # [DFF, C]

    f8l = ml_dtypes.float8_e4m3
    qk_dt = f8l if FP8_QK else bfl
    qk_s = S_QK if FP8_QK else 1.0

    def pmaj(a, nct):
        """[X, (ct p), d] -> partition-major [X, p, ct, d] (contiguous per-p
        DMA lines)"""
        x, cpd, dd = a.shape
        return np.ascontiguousarray(
            a.reshape(x, nct, 128, dd).transpose(0, 2, 1, 3))

    wq_l = pmaj((Wq * qk_s).reshape(NP, 2, C, D).transpose(0, 2, 1, 3)
                .reshape(NP, C, 128), NCT).astype(qk_dt)
    wk_l = pmaj((Wk * qk_s).reshape(NP, 2, C, D).transpose(0, 2, 1, 3)
                .reshape(NP, C, 128), NCT).astype(qk_dt)
    wv_l = pmaj((Wv * (S_V if FP8_V else 1.0)).reshape(4, 4, C, D)
                .transpose(0, 2, 1, 3).reshape(4, C, 256),
                NCT).astype(f8l if FP8_V else bfl)
    # wo: [NCT, (half ci p), d] -> [NCT, half, p, ci, d]
    wo_l = np.ascontiguousarray(
        Wo.reshape(C, NCT, 128).transpose(1, 0, 2)
        .reshape(NCT, 2, 4, 128, 128).transpose(0, 1, 3, 2, 4)).astype(bfl)
    w1s = (w1 * S1).reshape(C, 32, 128).transpose(1, 0, 2)  # [32, C, 128]
    if FP8_GATE_FULL:
        w1f_l = pmaj(w1s, NCT).astype(f8l)
        w1a_l = w1b_l = None
    else:
        w1a_l = pmaj(np.ascontiguousarray(w1s[:, :C // 2]),
                     4).astype(f8l if FP8_GATE4 else bfl)
        w1b_l = pmaj(np.ascontiguousarray(w1s[:, C // 2:]), 4).astype(bfl)
    w2_l = pmaj((w2 * (S2 if FP8_VAL else 1.0)).reshape(C, 32, 128)
                .transpose(1, 0, 2), NCT).astype(f8l if FP8_VAL else bfl)
    # w3: [NCT, (dc ft p), d] -> [NCT, dc, p, ft, d]
    w3_l = np.ascontiguousarray(
        (w3 * (S3 if FP8_GVW3 else 1.0)).reshape(DFF, NCT, 128)
        .transpose(1, 0, 2).reshape(NCT, 4, 8, 128, 128)
        .transpose(0, 1, 3, 2, 4)).astype(f8l if FP8_GVW3 else bfl)

    def packp(v):
        return np.ascontiguousarray(np.asarray(v, np.float32).reshape(-1, 128).T)

    cvec_l = np.ascontiguousarray(np.stack(
        [packp(inputs['g1']), packp(inputs['be1']), packp(inputs['g2']),
         packp(inputs['be2']), packp(inputs['bo']), packp(inputs['b3'])],
        axis=1))
    # b2 is added to pvl, which carries the S2 weight scale
    bvec_l = np.ascontiguousarray(np.stack(
        [packp(inputs['b1']),
         packp(np.asarray(inputs['b2'], np.float32) * (S2 if FP8_VAL else 1.0))],
        axis=1))

    mask_tiles = make_mask_tiles(mask, causal)

    in_maps = []
    for core in range(8):
        b, par = core // 2, core % 2
        xt_c = np.ascontiguousarray(x[b].T)            # [C, T]
        if par == 1:
            xt_c = np.ascontiguousarray(swap_cols64(xt_c))
        im = dict(
            xt=xt_c, maskt=mask_tiles[core],
            wq=wq_l, wk=wk_l, wv=wv_l, wo=wo_l, w2=w2_l, w3=w3_l,
            cvec=cvec_l, bvec=bvec_l)
        im['ident'] = np.eye(128, dtype=bfl)
        if FP8_GATE_FULL:
            im['w1f'] = w1f_l
        else:
            im['w1a'] = w1a_l
            im['w1b'] = w1b_l
        in_maps.append(im)
    szflags = dict(
        ln1=bool(np.all(np.asarray(inputs['g1']) == 1)
                 and np.all(np.asarray(inputs['be1']) == 0)),
        ln2=bool(np.all(np.asarray(inputs['g2']) == 1)
                 and np.all(np.asarray(inputs['be2']) == 0)),
        bo=bool(np.all(np.asarray(inputs['bo']) == 0)),
        b1=bool(np.all(np.asarray(inputs['b1']) == 0)),
        b2=bool(np.all(np.asarray(inputs['b2']) == 0)),
        b3=bool(np.all(np.asarray(inputs['b3']) == 0)),
    )
    return in_maps, causal, szflags


def assemble(outs, B=4):
    """outs: list of 8 per-core dicts with 'out' [C, TOK] -> [B, T, C]"""
    full = np.zeros((B, T, C), np.float32)
    for core in range(8):
        b, par = core // 2, core % 2
        o = np.asarray(outs[core]['out']).reshape(C, NCH, 64)
        for p in range(NCH):
            j = 2 * p + par
            full[b, 64 * j:64 * j + 64, :] = o[:, p, :].T
    return full


# ===================== entry point =====================

_NC_CACHE = {}


def _get_nc(causal, sz):
    key = (causal, tuple(sorted(sz.items())))
    if key not in _NC_CACHE:
        _NC_CACHE[key] = build_nc(causal=causal, sz=sz, silu_act=True)
    return _NC_CACHE[key]


def run_on_hw(inputs):
    from concourse import bass2jax
    in_maps, causal, sz = prep_in_maps(inputs)
    nc = _get_nc(causal, sz)
    results = bass2jax.run_bass_via_pjrt(nc, in_maps, n_cores=8)
    return assemble(results)


def kernel(**inputs):
    return run_on_hw(inputs)



# revision 68
# speedup vs baseline: 1.0052x; 1.0052x over previous
"""Self-contained TRN2 kernel for nn_Block_41695542510261 (dense transformer block).

Accepts FULL unsharded inputs, distributes across 8 NeuronCores internally
(2 cores per batch element, causal-balanced 64-row query chunks), returns
the FULL [4, 1024, 1024] output. See build_nc docstring for the design.
"""
import sys, os
for _p in ('/opt/trn_rl_repo', '/root/.axon_site/_ro/trn_rl_repo'):
    if os.path.isdir(_p) and _p not in sys.path:
        sys.path.insert(0, _p)
"""Transformer block kernel for TRN2 — 8-core SPMD, feature-major layout.

Reference: pre-LN attention block + SwiGLU FFN, B=4 T=1024 C=1024 H=16 D=64 DFF=4096.

Sharding: core c handles batch b=c//2, parity par=c%2. Each batch's 16
64-row query chunks split by parity: position p=0..7 <-> chunk j=2p+par.
Causal key-tile count for position p is p+1 for BOTH parities, so one
uniform SPMD program serves all 8 cores. Odd cores receive x with
adjacent 64-column blocks swapped so "own" tokens always sit at even
block positions (compile-time APs stay uniform); key order inside each
128-key tile is permuted consistently for K/V/mask, which attention
sums are invariant to.

Layout: all activations feature-major (xT[c, t]). LN stats via
ones-matmul over the partition (channel) dim + PE outer-product
broadcast. Attention computes S^T = (q.k)^T directly (lhsT=kT, rhs=qT),
softmax without max subtraction (scores bounded; scale 1/32 applied in
the exp), causal masking via 0/1 multiply on the single diagonal key
tile, denominator via a ones-column appended to V, normalization via a
K=1 outer-product matmul (hi/lo split for near-fp32 precision).

Matmuls run in bf16 with fp32 PSUM accumulation, except six fp8
(e4m3, DoubleRow = 2x PE rate) conversions chosen via a numpy
quantization sim validated against measured hw error (sim tracks hw
within ~0.1e-2): Wq/Wk projections, the V projection (its output is
fp8 for AV anyway, so ~free), the FFN val path (w2), gv@w3, P/V in
the attention AV matmul (kt-pair DoubleRow), and half the w1 gate
contraction (full-fp8 gate and fp8 Wo both measured over the 2e-2
gate). Weights are host-scaled by powers of 2 to clear fp8
subnormals; scales divide out in the exp scale / activation scale /
output scale. Weights are also host-repacked partition-major so DMA
partition lines are 1-2KB contiguous (128B lines ran the DMA engine
at ~40GB/s and stalled the w3 phase). Wo's first ci-half runs during
late attention as real filler. The residual path stays fp32.

Reciprocals (LN rsqrt and the softmax denominator) run on the scalar
engine as exp(-ln(x)) / exp(-0.5 ln(x)) (~5e-5 rel, measured): the
DVE RECIPROCAL (1.8-3.3us, free-dim-serial) had been the dominant
PE-stall edge via the score-tile PSUM ring. Ln/Exp/Square share one
act table set; the Silu set is preloaded via a dummy activation
data-pinned after LN2's Exp (a no-dep activation gets hoisted by the
scheduler and forces two extra table swaps).

LN chains write their quantized consumers directly (fast8 path):
bf16-quality mu/rs broadcasts suffice because every consumer is fp8
or bf16-quantized, and the 2-byte vector normalize chain halves the
serial cost. The stats ones-vector carries 1/C so stats matmuls
produce mu / E[x^2] directly.

keep_warm matmuls hold the HAM activity clock at k=8 (k=4 halves the
PE clock; idle quanta trigger it). Warm blocks that should fill a
specific stall are data-pinned (keep_warm_on) to a just-produced
tile; the scheduler hoists dependency-free matmuls away from their
emission point. 256-col warms double LDWEIGHTS overhead (~+27us) —
keep 512-col in hot paths.
"""
import contextlib
import json
import numpy as np
import ml_dtypes

import concourse.bass as bass
import concourse.mybir as mybir
import concourse.tile as tile

f32 = mybir.dt.float32
bf16 = mybir.dt.bfloat16
f8 = mybir.dt.float8e4
AF = mybir.ActivationFunctionType
DR = mybir.MatmulPerfMode.DoubleRow

C = 1024        # d_model
T = 1024        # seq len
H = 16          # heads
D = 64          # head dim
DFF = 4096
TOK = 512       # own tokens per core
NCT = C // 128  # 8 c tiles
NTT = T // 128  # 8 token (key) tiles
NP = H // 2     # 8 head pairs
NCH = 8         # q chunks per core (64 rows each)
EPS = 1e-5

# fp8 (e4m3, DoubleRow 2x matmul) coverage. Error budget per numpy sim:
# qk is ~free (softmax washes it out), val + gvw3 together land ~1.5e-2
# max-rel vs the 2e-2 gate. v/gate/ctxwo stay bf16 (worst err/perf ratio).
FP8_QK = True    # Wq/Wk projections (h, weights fp8)
FP8_VAL = True   # FFN w2 (val) path
FP8_GVW3 = True  # gv -> w3 matmul
FP8_AV = True    # P (exp out), v_all, mask in fp8; AV via DoubleRow kt pairs
FP8_GATE4 = True # FFN w1 (gate): first 4 of 8 ct tiles fp8-DR, rest bf16
FP8_GATE_FULL = False  # FFN w1 fully fp8-DR (hw: 2.096e-2, over gate)
FP8_V = True     # V projection fp8-DR (v_all is fp8 anyway: ~free error)
LN_FAST = True   # bf16 mu/rs LN chains writing fp8/bf16 dst directly
DEN_BUFS = 0     # >0: pden gets its own PSUM pool (sps shrinks to 2)
S1 = 256.0       # host scale on w1 (fp8: clears subnormals; |w1*S1|max ~12 << 448)
S_QK = 64.0      # host scale on Wq/Wk (w~0.02 must clear fp8 subnormals)
S_V = 64.0       # host scale on Wv (fp8)
S2 = 32.0        # host scale on w2 (keeps |gv*S2| < 240; 128 overflows)
S3 = 256.0       # host scale on w3 (fp8 subnormal clearance)
PO_SCALE = 1.0 / (S2 * S3)
SM_SCALE = (1.0 / 32.0) / (S_QK * S_QK if FP8_QK else 1.0)  # 1/sqrt(d_model)


def split_multiwaits(bir_bytes: bytes) -> bytes:
    """Split multi-wait instructions into single-wait EventSemaphore
    carriers placed just before them on the same engine. This walrus
    build has one sync-wait slot for several ISA structs (self-loading
    matmuls, direct DMAs, drains)."""
    m = json.loads(bir_bytes)
    ctr = 0
    for f in m['functions']:
        for blk in f.get('blocks', []):
            insts = blk.get('instructions', [])
            out = []
            changed = False
            for i in insts:
                si = i.get('sync_info')
                w = (si or {}).get('on_wait') or []
                if len(w) > 1:
                    for extra in w[:-1]:
                        ctr += 1
                        out.append({
                            'debug': i.get('debug'),
                            'engine': i['engine'],
                            'ins': [], 'outs': [],
                            'name': f'I-esw-{ctr}',
                            'opcode': 'EventSemaphore',
                            'sync_info': {'on_update': [], 'on_wait': [extra]},
                        })
                    si['on_wait'] = [w[-1]]
                    changed = True
                out.append(i)
            if changed:
                blk['instructions'] = out
    return json.dumps(m).encode()


def patch_nc(nc):
    orig = nc.to_json_bytes
    nc.to_json_bytes = lambda: split_multiwaits(orig())
    return nc


def build_nc(causal=True, sz=None, silu_act=True):
    sz = sz or {}
    nc = bass.Bass(trn_type="TRN2", target_bir_lowering=False, debug=False)

    xt = nc.dram_tensor("xt", [C, T], f32, kind="ExternalInput")
    av_dt = f8 if FP8_AV else bf16
    if causal:
        maskt = nc.dram_tensor("maskt", [128, 2, 128], av_dt, kind="ExternalInput")
    else:
        maskt = nc.dram_tensor("maskt", [4, NTT, 128, 128], av_dt, kind="ExternalInput")
    qk_dt = f8 if FP8_QK else bf16
    w2_dt = f8 if FP8_VAL else bf16
    w3_dt = f8 if FP8_GVW3 else bf16
    # weights arrive pre-transposed to partition-major [.., 128p, ..] so each
    # DMA partition line is 1-2KB contiguous (128B lines ran the DMA engine
    # at ~40GB/s and stalled the w3 phase)
    wq = nc.dram_tensor("wq", [NP, 128, NCT, 128], qk_dt, kind="ExternalInput")
    wk = nc.dram_tensor("wk", [NP, 128, NCT, 128], qk_dt, kind="ExternalInput")
    wv = nc.dram_tensor("wv", [4, 128, NCT, 256], f8 if FP8_V else bf16,
                        kind="ExternalInput")
    wo = nc.dram_tensor("wo", [NCT, 2, 128, 4, 128], bf16, kind="ExternalInput")
    if FP8_GATE_FULL:
        w1f = nc.dram_tensor("w1f", [32, 128, NCT, 128], f8, kind="ExternalInput")
        w1a = w1b = None
    else:
        w1a_dt = f8 if FP8_GATE4 else bf16
        w1a = nc.dram_tensor("w1a", [32, 128, 4, 128], w1a_dt, kind="ExternalInput")
        w1b = nc.dram_tensor("w1b", [32, 128, 4, 128], bf16, kind="ExternalInput")
    w2 = nc.dram_tensor("w2", [32, 128, NCT, 128], w2_dt, kind="ExternalInput")
    w3 = nc.dram_tensor("w3", [NCT, 4, 128, 8, 128], w3_dt, kind="ExternalInput")
    # packed per-channel constants: one DMA instead of eight (each
    # DMA_DIRECT2D costs ~650ns of sync-queue issue time at startup, which
    # delayed the critical xT input fetch)
    cvec = nc.dram_tensor("cvec", [128, 6, NCT], f32, kind="ExternalInput")
    bvec = nc.dram_tensor("bvec", [128, 2, 32], f32, kind="ExternalInput")
    ident = nc.dram_tensor("ident", [128, 128], bf16, kind="ExternalInput")
    out = nc.dram_tensor("out", [C, TOK], f32, kind="ExternalOutput")

    def cnt(p):
        return (p + 1) if causal else NTT

    with tile.TileContext(nc) as tc, contextlib.ExitStack() as ctx:
        consts = ctx.enter_context(tc.tile_pool(name="consts", bufs=1))
        perB = ctx.enter_context(tc.tile_pool(name="perB", bufs=1))
        w12p = ctx.enter_context(tc.tile_pool(name="w12", bufs=6))
        w3p = ctx.enter_context(tc.tile_pool(name="w3_sb", bufs=8))

        # ---- constants ----
        ones_row = consts.tile([1, 128], bf16)
        nc.vector.memset(ones_row, 1.0)
        ones_col = consts.tile([128, 1], bf16)
        # carries the 1/C stats normalization (2^-10, exact in bf16): the
        # stats matmuls then produce mu / E[x^2] directly, removing two
        # serial scalar muls from every LN finish chain
        nc.vector.memset(ones_col, 1.0 / C)
        eps_t = consts.tile([1, 1], f32)
        nc.vector.memset(eps_t, EPS)
        warm_t = consts.tile([128, 512], bf16)
        nc.vector.memset(warm_t, 0.0)
        warm_t8 = consts.tile([128, 128], f8)
        nc.vector.memset(warm_t8, 0.0)
        warm_tf = consts.tile([128, 128], f32)
        nc.vector.memset(warm_tf, 0.0)

        def keep_warm(pool, n, cols=512):
            # dependency-free matmuls that execute during upcoming PE
            # dependency stalls, keeping the HAM clock gate up; smaller
            # cols = finer granularity = less real-work delay
            wp = pool.tile([128, 512], f32, tag="wp")
            for _ in range(n):
                nc.tensor.matmul(wp[:, 0:cols], lhsT=warm_t[:, 0:128],
                                 rhs=warm_t[:, 0:cols], start=True, stop=True)

        def keep_warm_on(pool, n, rhs):
            # dep-pinned warm: the scheduler hoists dependency-free matmuls
            # away from the stall they're meant to fill; reading a
            # just-produced tile anchors them at the right spot
            cols = rhs.free_size()
            wp = pool.tile([128, 512], f32, tag="wp")
            # f32 rhs is NOT supported here: an f32 warm matmul corrupted
            # downstream fp8 results on this hw (PE mode interaction)
            assert rhs.dtype != f32
            lhs = warm_t8 if rhs.dtype == f8 else warm_t
            for _ in range(n):
                nc.tensor.matmul(wp[:, 0:cols], lhsT=lhs[:, 0:128],
                                 rhs=rhs, start=True, stop=True)

        # ---- phase-B persistent tiles (live to the end) ----
        x2T = perB.tile([128, NCT, TOK], f32)
        ln2_fast = LN_FAST and sz.get('ln2', False)
        x2b = None
        if ln2_fast:
            # bf16 cast of x2 (stats side-product) feeds the 2-byte LN2 chain
            x2b = perB.tile([128, NCT, TOK], bf16, tag="x2b")
        h2T = None
        if not FP8_GATE_FULL:
            h2T = perB.tile([128, NCT, TOK], bf16, tag="h2T")
        if FP8_VAL or FP8_GATE4 or FP8_GATE_FULL:
            h2q = perB.tile([128, NCT, TOK], f8)
        else:
            h2q = h2T
        outT = perB.tile([128, NCT, TOK], f32)

        # own-token columns (even 64-blocks) of [:, ct, :]
        def own(tl, ct):
            return tl[:, ct, :].rearrange(
                "p (j two i) -> p j two i", two=2, i=64)[:, :, 0, :]

        # ---------- feature-major layer norm ----------
        def ln_begin(stps, ntok):
            mean_ps = stps.tile([1, ntok], f32, tag="mean")
            ex2_ps = stps.tile([1, ntok], f32, tag="ex2")
            return mean_ps, ex2_ps

        def ln_stats_prep(src_ap, ntok, sqp, xb_dst=None):
            if xb_dst is None:
                xb = sqp.tile([128, ntok], bf16, tag="xb")
            else:
                xb = xb_dst
            nc.vector.tensor_copy(out=xb, in_=src_ap)
            sq = sqp.tile([128, ntok], bf16, tag="sq")
            # square on the scalar engine: vector is the busier queue here
            nc.scalar.activation(sq, xb, AF.Square)
            return xb, sq

        def ln_stats_mm(stats, prep, ct):
            # emitted a couple of cts behind the prep so the in-order PE
            # stream never waits on the vector/scalar prep chain
            mean_ps, ex2_ps = stats
            xb, sq = prep
            nc.tensor.matmul(mean_ps, lhsT=ones_col, rhs=xb,
                             start=(ct == 0), stop=(ct == NCT - 1))
            nc.tensor.matmul(ex2_ps, lhsT=ones_col, rhs=sq,
                             start=(ct == 0), stop=(ct == NCT - 1))

        # src(ct) -> [128, ntok] f32; writes dst(ct) (bf16) or fast8 dst8
        def layer_norm_T(src, dst, ntok, gs, bes, skip_affine,
                         sqp, stps, stss, bcp, tmpp, fast8=None):
            nh = ntok // 512
            assert nh == 1
            stats = ln_begin(stps, ntok)
            preps = []
            for ct in range(NCT):
                xbd = fast8['xb'](ct) if fast8 else None
                preps.append(ln_stats_prep(src(ct), ntok, sqp, xb_dst=xbd))
                if len(preps) >= 3:
                    ct_mm = ct - 2
                    ln_stats_mm(stats, preps[ct_mm], ct_mm)
            for ct_mm in (NCT - 2, NCT - 1):
                ln_stats_mm(stats, preps[ct_mm], ct_mm)
            ln_finish(stats, src, dst, ntok, gs, bes, skip_affine,
                      stss, bcp, tmpp, fast8=fast8)

        def ln_finish(stats, src, dst, ntok, gs, bes, skip_affine,
                      stss, bcp, tmpp, wp_pool=None, n_warm=24, fast8=None):
            mean_ps, ex2_ps = stats
            if fast8 is not None:
                keep_warm_on(wp_pool or bcp, n_warm, fast8['xb'](NCT - 2))
            else:
                keep_warm(wp_pool or bcp, n_warm)
            mu = mean_ps  # ones_col carries 1/C: PSUM rows are mu / E[x^2]
            musq = stss.tile([1, ntok], f32, tag="musq")
            nc.scalar.activation(musq, mean_ps, AF.Square)
            var = stss.tile([1, ntok], f32, tag="var")
            nc.vector.tensor_sub(var, ex2_ps, musq)
            # rs = exp(-0.5*ln(var+eps)) on the scalar engine: ~3e-5 rel
            # (measured), replaces Sqrt + the 3.3us single-partition DVE
            # reciprocal on the critical LN chain
            lnv = stss.tile([1, ntok], f32, tag="lnv")
            nc.scalar.activation(lnv, var, AF.Ln, bias=eps_t)
            rs = stss.tile([1, ntok], f32, tag="rs")
            nc.scalar.activation(rs, lnv, AF.Exp, scale=-0.5)
            mu_hi = stss.tile([1, ntok], bf16, tag="mu_hi")
            nc.vector.tensor_copy(out=mu_hi, in_=mu)
            if fast8 is None:
                rs_hi = stss.tile([1, ntok], bf16, tag="rs_hi")
                nc.vector.tensor_copy(out=rs_hi, in_=rs)
            if fast8 is not None:
                # consumers are fp8 (or bf16-quantized anyway): bf16-quality
                # mu/rs suffice, so use single-pass broadcasts and a 2-byte
                # vector normalize chain writing the quantized dst directly
                # (halves the serial chain and removes the cast trail that
                # gated the first downstream matmul)
                assert skip_affine
                mu_bc = bcp.tile([128, ntok], f32, tag="mu_bc")
                rs_bc = bcp.tile([128, ntok], f32, tag="rs_bc")
                nc.tensor.matmul(mu_bc, lhsT=ones_row, rhs=mu_hi,
                                 start=True, stop=True)
                # redundant mu-broadcasts bridge the PE over the Ln/Exp wait
                wpb = (wp_pool or bcp).tile([128, 512], f32, tag="wp")
                for _ in range(2):
                    nc.tensor.matmul(wpb[:, 0:ntok], lhsT=ones_row, rhs=mu_hi,
                                     start=True, stop=True)
                rs_hi = stss.tile([1, ntok], bf16, tag="rs_hi")
                nc.vector.tensor_copy(out=rs_hi, in_=rs)
                nc.tensor.matmul(rs_bc, lhsT=ones_row, rhs=rs_hi,
                                 start=True, stop=True)
                mu_sb = stss.tile([128, ntok], bf16, tag="mu_sb")
                nc.scalar.copy(out=mu_sb, in_=mu_bc)
                rs_sb = stss.tile([128, ntok], bf16, tag="rs_sb")
                nc.scalar.copy(out=rs_sb, in_=rs_bc)
                pw_n, pw_cols = fast8.get('post_warm', (12, 256))
                keep_warm_on(wp_pool or bcp, pw_n, mu_sb[:, 0:pw_cols])
                if fast8.get('preload_silu'):
                    # pull the Silu table swap off the first FFN activation:
                    # load it now, hidden behind the vector normalize chain.
                    # input dep on rs pins it AFTER the Ln/Exp pair (a no-dep
                    # activation gets hoisted by the scheduler and forces two
                    # extra table swaps)
                    dummy = stss.tile([1, 1], f32, tag="dummy")
                    nc.scalar.activation(dummy, rs[0:1, 0:1], AF.Silu)
                for ct in range(NCT):
                    # fp8-out mul costs 1.37us vs 0.41us bf16 (fast DVE mode
                    # lost), but skipping the bf16 intermediate is worth
                    # ~0.24e-2 of the error budget (double rounding)
                    tmpb = tmpp.tile([128, ntok], bf16, tag="lntmpb")
                    nc.vector.tensor_sub(tmpb, fast8['xb'](ct), mu_sb)
                    nc.vector.tensor_mul(fast8['dst8'](ct), tmpb, rs_sb)
                return
            # hi/lo split of mu and rs for near-fp32 broadcast
            mu_lob = stss.tile([1, ntok], bf16, tag="mu_lob")
            nc.vector.tensor_sub(mu_lob, mu, mu_hi)
            rs_lob = stss.tile([1, ntok], bf16, tag="rs_lob")
            nc.vector.tensor_sub(rs_lob, rs, rs_hi)
            mu_bc = bcp.tile([128, ntok], f32, tag="mu_bc")
            rs_bc = bcp.tile([128, ntok], f32, tag="rs_bc")
            nc.tensor.matmul(mu_bc, lhsT=ones_row, rhs=mu_hi,
                             start=True, stop=False)
            nc.tensor.matmul(mu_bc, lhsT=ones_row, rhs=mu_lob,
                             start=False, stop=True)
            nc.tensor.matmul(rs_bc, lhsT=ones_row, rhs=rs_hi,
                             start=True, stop=False)
            nc.tensor.matmul(rs_bc, lhsT=ones_row, rhs=rs_lob,
                             start=False, stop=True)
            for ct in range(NCT):
                tmp = tmpp.tile([128, ntok], f32, tag="lntmp")
                nc.vector.tensor_sub(tmp, src(ct), mu_bc)
                if skip_affine:
                    nc.vector.tensor_mul(dst(ct), tmp, rs_bc)
                else:
                    nc.vector.tensor_mul(tmp, tmp, rs_bc)
                    nc.scalar.activation(dst(ct), tmp, AF.Identity,
                                         bias=bes[:, ct:ct + 1],
                                         scale=gs[:, ct:ct + 1])

        with tc.tile_pool(name="perA", bufs=1) as perA:
            # ---- phase-A persistent tiles ----
            xT = perA.tile([128, NCT, T], f32)
            if LN_FAST and FP8_QK and FP8_V and sz.get('ln1', False):
                # h exists only as fp8; xbT (bf16 cast of x, stats
                # side-product) feeds the 2-byte normalize chain
                xbT = perA.tile([128, NCT, T], bf16)
                hT = None
                hq = perA.tile([128, NCT, T], f8)
            else:
                xbT = None
                hT = perA.tile([128, NCT, T], bf16)
                if FP8_QK:
                    hq = perA.tile([128, NCT, T], f8)
                else:
                    hq = hT
            v_all = perA.tile([128, NTT, H, 65], av_dt)
            ctxT = perA.tile([128, NCT, TOK], bf16)
            wo_all = perA.tile([128, NCT, 4, 128], bf16)
            x2h1 = perA.tile([128, NCT, TOK], bf16)

            for th in range(2):
                for ct in range(NCT):
                    nc.sync.dma_start(
                        out=xT[:, ct, th * 512:(th + 1) * 512],
                        in_=xt[ct * 128:(ct + 1) * 128, th * 512:(th + 1) * 512])
            # constants issued after the critical xT input stream
            cv = consts.tile([128, 6, NCT], f32)
            nc.sync.dma_start(out=cv, in_=cvec[:, :, :])
            g1s, be1s, g2s, be2s, bos, b3s = (cv[:, i, :] for i in range(6))
            bv = consts.tile([128, 2, 32], f32)
            nc.sync.dma_start(out=bv, in_=bvec[:, :, :])
            id_sb = consts.tile([128, 128], bf16)
            nc.sync.dma_start(out=id_sb, in_=ident[:, :])
            b1s = bv[:, 0, :]
            b2s = bv[:, 1, :]
            if causal:
                mk = consts.tile([128, 2, 128], av_dt)
                nc.sync.dma_start(out=mk, in_=maskt[:, :, :])
            else:
                mk = consts.tile([128, 4, NTT, 128], av_dt)
                nc.sync.dma_start(
                    out=mk, in_=maskt[:, :, :, :].rearrange("c k p q -> p c k q"))

            with tc.tile_pool(name="ln_sq", bufs=3) as sqp, \
                 tc.tile_pool(name="ln_st", bufs=1, space="PSUM") as stps, \
                 tc.tile_pool(name="ln_sts", bufs=1) as stss, \
                 tc.tile_pool(name="ln_bc", bufs=1, space="PSUM") as bcp, \
                 tc.tile_pool(name="ln_tmp", bufs=2) as tmpp, \
                 tc.tile_pool(name="wv_sb", bufs=1) as wvp, \
                 tc.tile_pool(name="v_ps", bufs=2, space="PSUM") as vps:
                # prefetch both V weight halves up front
                wv_dt = f8 if FP8_V else bf16
                wv_sbs = []
                for g in range(2):
                    wv_sb = wvp.tile([128, NCT, 2, 256], wv_dt, tag=f"wv{g}")
                    for q in range(2):
                        nc.sync.dma_start(
                            out=wv_sb[:, :, q, :], in_=wv[2 * g + q])
                    wv_sbs.append(wv_sb)

                def v_block(tts):
                    for g in range(2):
                        for tt in tts:
                            pv = vps.tile([128, 512], f32, tag="pv")
                            if FP8_V:
                                for g2 in range(4):
                                    nc.tensor.matmul(
                                        pv,
                                        lhsT=hq[:, 2 * g2:2 * g2 + 2,
                                                tt * 128:(tt + 1) * 128],
                                        rhs=wv_sbs[g][:, 2 * g2:2 * g2 + 2, :, :],
                                        perf_mode=DR,
                                        start=(g2 == 0), stop=(g2 == 3))
                                nc.scalar.activation(
                                    v_all[:, tt, 8 * g:8 * (g + 1), 0:64],
                                    pv[:, :].rearrange("p (h d) -> p h d", d=64),
                                    AF.Copy, scale=1.0 / S_V)
                            else:
                                for ct in range(NCT):
                                    nc.tensor.matmul(
                                        pv, lhsT=hT[:, ct, tt * 128:(tt + 1) * 128],
                                        rhs=wv_sbs[g][:, ct, :, :],
                                        start=(ct == 0), stop=(ct == NCT - 1))
                                nc.scalar.copy(
                                    out=v_all[:, tt, 8 * g:8 * (g + 1), 0:64],
                                    in_=pv[:, :].rearrange("p (h d) -> p h d", d=64))

                # LN half 0 -> V for its token tiles fills LN half 1's
                # dependency stall with real matmuls; then LN half 1 -> rest
                for th in range(2):
                    tsl = slice(th * 512, (th + 1) * 512)
                    if xbT is not None:
                        layer_norm_T(lambda ct: xT[:, ct, tsl], None, 512,
                                     g1s, be1s, sz.get('ln1', False),
                                     sqp, stps, stss, bcp, tmpp,
                                     fast8=dict(
                                         xb=lambda ct: xbT[:, ct, tsl],
                                         dst8=lambda ct: hq[:, ct, tsl],
                                         post_warm=(10, 256)))
                    else:
                        layer_norm_T(lambda ct: xT[:, ct, tsl],
                                     lambda ct: hT[:, ct, tsl], 512,
                                     g1s, be1s, sz.get('ln1', False),
                                     sqp, stps, stss, bcp, tmpp)
                    if FP8_QK and xbT is None and FP8_V:
                        # fp8 V consumes hq: cast before v_block
                        for ct in range(NCT):
                            nc.scalar.copy(out=hq[:, ct, tsl],
                                           in_=hT[:, ct, tsl])
                    v_block(range(4 * th, 4 * th + 4))
                    # after v_block so the casts don't block V evacuation
                    # on the in-order scalar queue
                    if FP8_QK and xbT is None and not FP8_V:
                        for ct in range(NCT):
                            nc.scalar.copy(out=hq[:, ct, tsl],
                                           in_=hT[:, ct, tsl])
            nc.vector.memset(v_all[:, :, :, 64:65], 1.0)
            for cot in range(NCT):
                nc.sync.dma_start(
                    out=wo_all[:, cot, :, :],
                    in_=wo[cot, 0])

            # ---------- attention ----------
            with tc.tile_pool(name="wqk", bufs=2) as wqkp, \
                 tc.tile_pool(name="qk_ps", bufs=1, space="PSUM") as qkps, \
                 tc.tile_pool(name="qk_sb", bufs=2) as qksb, \
                 tc.tile_pool(name="s_ps", bufs=(2 if DEN_BUFS else 3),
                              space="PSUM") as sps, \
                 tc.tile_pool(name="p_sb", bufs=2) as psb, \
                 tc.tile_pool(name="ctx_ps", bufs=3, space="PSUM") as cps, \
                 tc.tile_pool(name="at_wp", bufs=1, space="PSUM") as wps, \
                 tc.tile_pool(name="nrm_sb", bufs=2) as nsb, \
                 contextlib.ExitStack() as dctx:
                dps = dctx.enter_context(
                    tc.tile_pool(name="den_ps", bufs=DEN_BUFS, space="PSUM")) \
                    if DEN_BUFS else None
                def emit_scores_both(qT, kT, P0, P1, m):
                    # merged mp pair {2m, 2m+1}: one 256-wide q block per
                    # matmul — halves the small-matmul count (the attention
                    # phase is per-instruction-overhead bound, ~150ns fixed
                    # cost on a 107ns stream). h2=0 rows 0:64, h2=1 rows
                    # 64:128 co-execute via PE row packing.
                    n_kt = cnt(4 * m + 3)
                    qsl = slice(m * 256, (m + 1) * 256)
                    for kg in range(0, n_kt, 2):
                        # the block at kg==4m+2 is causally dead for sub-mp
                        # 2m (cols 0:128): compute/exp only the valid half
                        # and memset the dead half (sole writer: no cross-
                        # engine WAW race, no serialization)
                        dead = causal and kg == 4 * m + 2
                        csl = slice(128, 256) if dead else slice(0, 256)
                        qs2 = slice(qsl.start + csl.start, qsl.start + csl.stop)
                        ps0 = sps.tile([128, 2, 256], f32, tag="ps_s")
                        ps1 = sps.tile([128, 2, 256], f32, tag="ps_s")
                        for kt in range(kg, kg + 2):
                            nc.tensor.matmul(
                                ps0[:, kt - kg, csl],
                                lhsT=kT[0:64, kt * 128:(kt + 1) * 128],
                                rhs=qT[0:64, qs2], start=True, stop=True)
                            nc.tensor.matmul(
                                ps1[:, kt - kg, csl],
                                lhsT=kT[64:128, kt * 128:(kt + 1) * 128],
                                rhs=qT[64:128, qs2], start=True, stop=True)
                        nc.scalar.activation(P0[:, kg:kg + 2, csl],
                                             ps0[:, :, csl],
                                             AF.Exp, scale=SM_SCALE)
                        nc.scalar.activation(P1[:, kg:kg + 2, csl],
                                             ps1[:, :, csl],
                                             AF.Exp, scale=SM_SCALE)
                        if dead:
                            nc.vector.memset(P0[:, kg:kg + 2, 0:128], 0.0)
                            nc.vector.memset(P1[:, kg:kg + 2, 0:128], 0.0)
                    for P in (P0, P1):
                        if causal:
                            # diag masks: sub-mp 2m on cols 0:128 (kts
                            # 4m..4m+1), sub-mp 2m+1 on cols 128:256
                            nc.vector.tensor_mul(
                                P[:, 4 * m:4 * m + 2, 0:128],
                                P[:, 4 * m:4 * m + 2, 0:128], mk)
                            nc.vector.tensor_mul(
                                P[:, 4 * m + 2:4 * m + 4, 128:256],
                                P[:, 4 * m + 2:4 * m + 4, 128:256], mk)

                        else:
                            nc.vector.tensor_mul(P[:, 0:n_kt, 0:128],
                                                 P[:, 0:n_kt, 0:128],
                                                 mk[:, 2 * m, 0:n_kt, :])
                            nc.vector.tensor_mul(P[:, 0:n_kt, 128:256],
                                                 P[:, 0:n_kt, 128:256],
                                                 mk[:, 2 * m + 1, 0:n_kt, :])

                def emit_av(P, pctx, hp, m, h2):
                    n_kt = cnt(4 * m + 3)
                    h = 2 * hp + h2
                    if FP8_AV and n_kt % 2 == 0:
                        # DoubleRow over kt pairs: halves AV matmul count
                        npair = n_kt // 2
                        for j in range(npair):
                            nc.tensor.matmul(
                                pctx[:, h2, :],
                                lhsT=v_all[:, 2 * j:2 * j + 2, h, :],
                                rhs=P[:, 2 * j:2 * j + 2, :], perf_mode=DR,
                                start=(j == 0), stop=(j == npair - 1))
                    else:
                        for kt in range(n_kt):
                            nc.tensor.matmul(
                                pctx[:, h2, :], lhsT=v_all[:, kt, h, :],
                                rhs=P[:, kt, :],
                                start=(kt == 0), stop=(kt == n_kt - 1))

                def emit_den(pctx):
                    # hi/lo bf16 split of the softmax denominator row so the
                    # PE ones-broadcast reconstructs it at ~fp32 in PSUM
                    d_hi = nsb.tile([1, 512], bf16, tag="d_hi")
                    nc.vector.tensor_copy(out=d_hi, in_=pctx[64:65, :, :])
                    d_lo = nsb.tile([1, 512], bf16, tag="d_lo")
                    nc.vector.tensor_sub(d_lo, pctx[64:65, :, :], d_hi)
                    return d_hi, d_lo

                def emit_norm_pair(ga, gb):
                    # normalize two merged groups (= 4 original mp groups)
                    # with ONE 1/den chain: A on partitions 0:64, B on 64:128
                    if DEN_BUFS:
                        pden = dps.tile([128, 512], f32, tag="pden")
                    else:
                        pden = sps.tile([128, 512], f32, tag="ps_s")
                    for row, g in ((0, ga), (64, gb)):
                        if g is None:
                            continue
                        _, d_hi, d_lo, _, _ = g
                        nc.tensor.matmul(pden[row:row + 64, :],
                                         lhsT=ones_row[0:1, 0:64], rhs=d_hi,
                                         start=True, stop=False)
                        nc.tensor.matmul(pden[row:row + 64, :],
                                         lhsT=ones_row[0:1, 0:64], rhs=d_lo,
                                         start=False, stop=True)
                    # 1/den = exp(-ln(den)) on the scalar engine (~5e-5 rel,
                    # measured): frees the pden ring after the quick Ln and
                    # keeps the DVE RECIPROCAL off the busy vector queue
                    lnd = nsb.tile([128, 512], f32, tag="lnd")
                    nrm2 = nsb.tile([128, 512], f32, tag="nrm2")
                    if gb is None:
                        nc.scalar.activation(lnd[0:64, :], pden[0:64, :], AF.Ln)
                        nc.scalar.activation(nrm2[0:64, :], lnd[0:64, :],
                                             AF.Exp, scale=-1.0)
                    else:
                        nc.scalar.activation(lnd, pden, AF.Ln)
                        nc.scalar.activation(nrm2, lnd, AF.Exp, scale=-1.0)
                    for row, g in ((0, ga), (64, gb)):
                        if g is None:
                            continue
                        pctx, _, _, php, pm = g
                        for h2 in range(2):
                            nc.vector.tensor_mul(
                                ctxT[64 * h2:64 * (h2 + 1), php,
                                     pm * 256:(pm + 1) * 256],
                                pctx[0:64, h2, :],
                                nrm2[row:row + 64, 256 * h2:256 * (h2 + 1)])

                # pipeline over merged groups: scores(g+1) are emitted before
                # av(g) so the in-order PE stream never drains while the
                # exp/mask chain runs; norm/evac for g trails one group.
                # qk projection emission, split so pair hp+1's dense
                # N=512 matmuls can be interleaved into pair hp's attention
                # groups (fills exp/mask bubbles in the in-order PE stream)
                def own2(tl, g):
                    # own-token (even 64-block) columns for ct pair 2g, 2g+1
                    return tl[:, 2 * g:2 * g + 2, :].rearrange(
                        "p c (j two i) -> p c j two i", two=2, i=64)[:, :, :, 0, :]

                def make_qk_parts(hp):
                    wq_sb = wqkp.tile([128, NCT, 128], qk_dt, tag="wq")
                    nc.sync.dma_start(
                        out=wq_sb, in_=wq[hp])
                    wk_sb = wqkp.tile([128, NCT, 128], qk_dt, tag="wk")
                    nc.sync.dma_start(
                        out=wk_sb, in_=wk[hp])
                    qT = qksb.tile([128, 512], bf16, tag="qT")
                    kT = qksb.tile([128, 1024], bf16, tag="kT")

                    def part_q():
                        pq = qkps.tile([128, 512], f32, tag="pqk")
                        if FP8_QK:
                            for g in range(4):
                                nc.tensor.matmul(pq, lhsT=wq_sb[:, 2 * g:2 * g + 2, :],
                                                 rhs=own2(hq, g), perf_mode=DR,
                                                 start=(g == 0), stop=(g == 3))
                        else:
                            for ct in range(NCT):
                                nc.tensor.matmul(pq, lhsT=wq_sb[:, ct, :],
                                                 rhs=own(hT, ct),
                                                 start=(ct == 0), stop=(ct == NCT - 1))
                        nc.scalar.copy(out=qT, in_=pq)

                    def part_k(hh):
                        sl = slice(hh * 512, (hh + 1) * 512)
                        pk = qkps.tile([128, 512], f32, tag="pqk")
                        if FP8_QK:
                            for g in range(4):
                                nc.tensor.matmul(pk, lhsT=wk_sb[:, 2 * g:2 * g + 2, :],
                                                 rhs=hq[:, 2 * g:2 * g + 2, sl],
                                                 perf_mode=DR,
                                                 start=(g == 0), stop=(g == 3))
                        else:
                            for ct in range(NCT):
                                nc.tensor.matmul(pk, lhsT=wk_sb[:, ct, :],
                                                 rhs=hT[:, ct, sl],
                                                 start=(ct == 0), stop=(ct == NCT - 1))
                        nc.scalar.copy(out=kT[:, sl], in_=pk)

                    return qT, kT, (part_q, lambda: part_k(0), lambda: part_k(1))

                prev = None          # (P0, P1, hp, m) awaiting av
                pend = []            # groups awaiting a paired norm
                wo_early = list(range(NCT))  # Wo ci 0..3 half, run as filler
                qT, kT, parts = make_qk_parts(0)
                for pf in parts:
                    pf()
                nxt = None
                for hp in range(NP):
                    if hp + 1 < NP:
                        nxt = make_qk_parts(hp + 1)
                    for m in range(2):
                        P0 = psb.tile([128, NTT, 256], av_dt, tag="P0")
                        P1 = psb.tile([128, NTT, 256], av_dt, tag="P1")
                        emit_scores_both(qT, kT, P0, P1, m)
                        if hp >= 5 and wo_early:
                            cot = wo_early.pop(0)
                            pa1 = wps.tile([128, TOK], f32, tag="wp")
                            for ci in range(4):
                                nc.tensor.matmul(pa1, lhsT=wo_all[:, cot, ci, :],
                                                 rhs=ctxT[:, ci, :],
                                                 start=(ci == 0), stop=(ci == 3))
                            nc.scalar.copy(out=x2h1[:, cot, :], in_=pa1)
                            keep_warm_on(wps, 4, P0[:, 0:2, :])
                        else:
                            keep_warm(wps, 4)
                            keep_warm_on(wps, 4, P0[:, 0:2, :])
                        if hp + 1 < NP:
                            if m == 0:
                                nxt[2][0]()  # pair hp+1 q projection filler
                            else:
                                nxt[2][1]()  # pair hp+1 k halves
                                nxt[2][2]()
                        if len(pend) == 2:
                            emit_norm_pair(pend[0], pend[1])
                            pend = []
                        if prev is not None:
                            pP0, pP1, php, pm = prev
                            pctx = cps.tile([65, 2, 256], f32, tag="pctx")
                            emit_av(pP0, pctx, php, pm, 0)
                            emit_av(pP1, pctx, php, pm, 1)
                            d_hi, d_lo = emit_den(pctx)
                            pend.append((pctx, d_hi, d_lo, php, pm))
                        prev = (P0, P1, hp, m)
                    if hp + 1 < NP:
                        qT, kT, _ = nxt[0], nxt[1], None
                # drain
                if prev is not None:
                    pP0, pP1, php, pm = prev
                    pctx = cps.tile([65, 2, 256], f32, tag="pctx")
                    emit_av(pP0, pctx, php, pm, 0)
                    emit_av(pP1, pctx, php, pm, 1)
                    d_hi, d_lo = emit_den(pctx)
                    pend.append((pctx, d_hi, d_lo, php, pm))
                while pend:
                    ga = pend.pop(0)
                    gb = pend.pop(0) if pend else None
                    emit_norm_pair(ga, gb)
                    keep_warm(wps, 4)
                while wo_early:
                    cot = wo_early.pop(0)
                    pa1 = wps.tile([128, TOK], f32, tag="wp")
                    for ci in range(4):
                        nc.tensor.matmul(pa1, lhsT=wo_all[:, cot, ci, :],
                                         rhs=ctxT[:, ci, :],
                                         start=(ci == 0), stop=(ci == 3))
                    nc.scalar.copy(out=x2h1[:, cot, :], in_=pa1)

            # ---------- Wo + residual, LN2 stats interleaved per cot ----------
            with tc.tile_pool(name="wo_sb", bufs=2) as wop, \
                 tc.tile_pool(name="a_ps", bufs=2, space="PSUM") as aps, \
                 tc.tile_pool(name="a_sb", bufs=2) as asb, \
                 tc.tile_pool(name="l2_sq", bufs=3) as sqp2, \
                 tc.tile_pool(name="l2_st", bufs=1, space="PSUM") as stps2, \
                 tc.tile_pool(name="l2_sts", bufs=1) as stss2, \
                 tc.tile_pool(name="l2_bc", bufs=1, space="PSUM") as bcp2, \
                 tc.tile_pool(name="l2_tmp", bufs=2) as tmpp2:
                keep_warm(aps, 6)
                stats2 = ln_begin(stps2, TOK)
                preps2 = []
                for cot in range(NCT):
                    wo_sb = wop.tile([128, 4, 128], bf16, tag="wo")
                    nc.sync.dma_start(
                        out=wo_sb,
                        in_=wo[cot, 1])
                    pa = aps.tile([128, TOK], f32, tag="pa")
                    for ci in range(4):
                        nc.tensor.matmul(pa, lhsT=wo_sb[:, ci, :],
                                         rhs=ctxT[:, 4 + ci, :],
                                         start=(ci == 0), stop=False)
                    # fold the x2h1 (Wo first-half partial) add into the
                    # PSUM via identity weights: +0.43us on the starving PE
                    # here buys back a 1.37us f32 vector add per cot
                    nc.tensor.matmul(pa, lhsT=id_sb, rhs=x2h1[:, cot, :],
                                     start=False, stop=True)
                    if sz.get('bo', False):
                        nc.vector.tensor_add(x2T[:, cot, :], pa, own(xT, cot))
                    else:
                        tmpa = asb.tile([128, TOK], f32, tag="tmpa")
                        nc.scalar.activation(tmpa, pa, AF.Identity,
                                             bias=bos[:, cot:cot + 1], scale=1.0)
                        nc.vector.tensor_add(x2T[:, cot, :], tmpa, own(xT, cot))
                    preps2.append(ln_stats_prep(
                        x2T[:, cot, :], TOK, sqp2,
                        xb_dst=(x2b[:, cot, :] if ln2_fast else None)))
                    if len(preps2) >= 3:
                        ln_stats_mm(stats2, preps2[cot - 2], cot - 2)
                for ct_mm in (NCT - 2, NCT - 1):
                    ln_stats_mm(stats2, preps2[ct_mm], ct_mm)
                if ln2_fast:
                    n8 = NCT if FP8_GATE_FULL else 4

                    def ln2_dst8(ct):
                        return h2q[:, ct, :] if ct < n8 else h2T[:, ct, :]
                    ln_finish(stats2, lambda ct: x2T[:, ct, :], None, TOK,
                              g2s, be2s, True, stss2, bcp2, tmpp2,
                              wp_pool=aps, n_warm=10,
                              fast8=dict(xb=lambda ct: x2b[:, ct, :],
                                         dst8=ln2_dst8,
                                         post_warm=(12, 256),
                                         preload_silu=True))
                    if FP8_VAL and not FP8_GATE_FULL:
                        for ct in range(n8, NCT):
                            nc.scalar.copy(out=h2q[:, ct, :], in_=h2T[:, ct, :])
                else:
                    ln_finish(stats2, lambda ct: x2T[:, ct, :],
                              lambda ct: h2T[:, ct, :], TOK, g2s, be2s,
                              sz.get('ln2', False), stss2, bcp2, tmpp2,
                              wp_pool=aps, n_warm=10)
                    if FP8_VAL or FP8_GATE_FULL:
                        for ct in range(NCT):
                            nc.scalar.copy(out=h2q[:, ct, :], in_=h2T[:, ct, :])

        # ---------- FFN ----------
        with tc.tile_pool(name="g_ps", bufs=2, space="PSUM") as gps, \
             tc.tile_pool(name="vl_ps", bufs=2, space="PSUM") as vlps, \
             tc.tile_pool(name="g_sb", bufs=2) as gsbp, \
             tc.tile_pool(name="gv_sb", bufs=1) as gvp, \
             tc.tile_pool(name="o_ps", bufs=2, space="PSUM") as ops:
            gv_all = gvp.tile([128, 4, 8, TOK], w3_dt, tag="gv")
            # lookahead w3 weight fetch: issue DMAs well before the w3 loop
            # so its matmuls never wait on HBM
            w3_tiles = {}
            w3_next = [0]

            def w3_fetch_upto(n):
                while w3_next[0] < min(n, 4 * NCT):
                    i = w3_next[0]
                    cot, dc = divmod(i, 4)
                    t3 = w3p.tile([128, 8, 128], w3_dt, tag="w3")
                    nc.sync.dma_start(
                        out=t3,
                        in_=w3[cot, dc])
                    w3_tiles[i] = t3
                    w3_next[0] += 1

            for dc in range(4):
                gv = gv_all[:, dc, :, :]
                for fi in range(8):
                    ft = dc * 8 + fi
                    if dc == 3:
                        w3_fetch_upto(fi)
                    if FP8_GATE_FULL:
                        w1_sb = w12p.tile([128, NCT, 128], f8, tag="w1f")
                        nc.sync.dma_start(
                            out=w1_sb,
                            in_=w1f[ft])
                    else:
                        w1a_sb = w12p.tile([128, 4, 128], w1a_dt, tag="w1a")
                        nc.sync.dma_start(
                            out=w1a_sb, in_=w1a[ft])
                        w1b_sb = w12p.tile([128, 4, 128], bf16, tag="w1b")
                        nc.sync.dma_start(
                            out=w1b_sb, in_=w1b[ft])
                    w2_sb = w12p.tile([128, NCT, 128], w2_dt, tag="w2")
                    nc.sync.dma_start(
                        out=w2_sb, in_=w2[ft])
                    pg = gps.tile([128, TOK], f32, tag="pg")
                    pvl = vlps.tile([128, TOK], f32, tag="pvl")
                    if FP8_GATE_FULL:
                        for g in range(4):
                            nc.tensor.matmul(pg, lhsT=w1_sb[:, 2 * g:2 * g + 2, :],
                                             rhs=h2q[:, 2 * g:2 * g + 2, :],
                                             perf_mode=DR,
                                             start=(g == 0), stop=(g == 3))
                    elif FP8_GATE4:
                        for g in range(2):
                            nc.tensor.matmul(pg, lhsT=w1a_sb[:, 2 * g:2 * g + 2, :],
                                             rhs=h2q[:, 2 * g:2 * g + 2, :],
                                             perf_mode=DR,
                                             start=(g == 0), stop=False)
                    else:
                        for ci in range(4):
                            nc.tensor.matmul(pg, lhsT=w1a_sb[:, ci, :],
                                             rhs=h2T[:, ci, :],
                                             start=(ci == 0), stop=False)
                    if not FP8_GATE_FULL:
                        for ci in range(4):
                            nc.tensor.matmul(pg, lhsT=w1b_sb[:, ci, :],
                                             rhs=h2T[:, 4 + ci, :],
                                             start=False, stop=(ci == 3))
                    if FP8_VAL:
                        for g in range(4):
                            nc.tensor.matmul(pvl, lhsT=w2_sb[:, 2 * g:2 * g + 2, :],
                                             rhs=h2q[:, 2 * g:2 * g + 2, :],
                                             perf_mode=DR,
                                             start=(g == 0), stop=(g == 3))
                    else:
                        for ct in range(NCT):
                            nc.tensor.matmul(pvl, lhsT=w2_sb[:, ct, :],
                                             rhs=h2T[:, ct, :],
                                             start=(ct == 0), stop=(ct == NCT - 1))
                    gs_t = gsbp.tile([128, TOK], f32, tag="gs_t")
                    if silu_act:
                        nc.scalar.activation(gs_t, pg, AF.Silu,
                                             bias=b1s[:, ft:ft + 1], scale=1.0 / S1)
                    else:
                        # silu(x) = x * sigmoid(x); x = pg + b1
                        nc.scalar.activation(gs_t, pg, AF.Sigmoid,
                                             bias=b1s[:, ft:ft + 1], scale=1.0 / S1)
                        if sz.get('b1', False):
                            nc.vector.tensor_mul(gs_t, gs_t, pg)
                        else:
                            xg = gsbp.tile([128, TOK], f32, tag="xg")
                            nc.vector.tensor_scalar_add(xg, pg, b1s[:, ft:ft + 1])
                            nc.vector.tensor_mul(gs_t, gs_t, xg)
                    if sz.get('b2', False):
                        nc.vector.tensor_mul(gv[:, fi, :], pvl, gs_t)
                    else:
                        nc.vector.tensor_scalar_add(gv[:, fi, :], pvl,
                                                    b2s[:, ft:ft + 1])
                        nc.vector.tensor_mul(gv[:, fi, :], gv[:, fi, :], gs_t)
            # cot-major w3: all 4 dc chunks accumulate in one PSUM group,
            # one scale+add per output tile (replaces 32 vector accumulates)
            po_s = (S2 if FP8_VAL else 1.0) * (S3 if FP8_GVW3 else 1.0)
            for cot in range(NCT):
                po = ops.tile([128, TOK], f32, tag="po")
                for dc in range(4):
                    w3_fetch_upto(4 * cot + dc + 7)
                    w3_sb = w3_tiles.pop(4 * cot + dc)
                    if cot == NCT - 1 and dc == 3:
                        # hold k=8 through the w3 drain: the final matmuls
                        # were running at half clock (k=4 fired ~2us early)
                        keep_warm_on(ops, 5, w3_sb[:, 0:4, :])
                    if FP8_GVW3:
                        for g in range(4):
                            nc.tensor.matmul(po, lhsT=w3_sb[:, 2 * g:2 * g + 2, :],
                                             rhs=gv_all[:, dc, 2 * g:2 * g + 2, :],
                                             perf_mode=DR,
                                             start=(dc == 0 and g == 0),
                                             stop=(dc == 3 and g == 3))
                    else:
                        for fi in range(8):
                            nc.tensor.matmul(po, lhsT=w3_sb[:, fi, :],
                                             rhs=gv_all[:, dc, fi, :],
                                             start=(dc == 0 and fi == 0),
                                             stop=(dc == 3 and fi == 7))
                if not sz.get('b3', False):
                    tmpo = gsbp.tile([128, TOK], f32, tag="tmpo")
                    nc.scalar.activation(tmpo, po, AF.Identity,
                                         bias=b3s[:, cot:cot + 1],
                                         scale=1.0 / po_s)
                    nc.vector.tensor_add(outT[:, cot, :], tmpo, x2T[:, cot, :])
                else:
                    nc.vector.scalar_tensor_tensor(
                        out=outT[:, cot, :], in0=po, scalar=1.0 / po_s,
                        in1=x2T[:, cot, :], op0=mybir.AluOpType.mult,
                        op1=mybir.AluOpType.add)
                nc.sync.dma_start(out=out[cot * 128:(cot + 1) * 128, :],
                                  in_=outT[:, cot, :])
    patch_nc(nc)
    return nc


# ===================== host-side prep =====================

def swap_cols64(a):
    """swap adjacent 64-col blocks along last axis"""
    s = a.shape
    b = a.reshape(*s[:-1], s[-1] // 128, 2, 64)
    return b[..., ::-1, :].reshape(s)


def check_causal(mask):
    T_ = mask.shape[0]
    allow = ~np.isneginf(np.asarray(mask))
    allow_ref = ~np.triu(np.ones((T_, T_), bool), k=1)
    return np.array_equal(allow, allow_ref)


def make_mask_tiles(mask, causal):
    """per-core multiplicative mask tiles (bf16 0/1), key-order swapped for odd cores.

    Merged q-chunk pairs: positions {2mp, 2mp+1} share one N=128 block.
    Causal: one [128, 2, 128] tile — [:,0,:] masks key tile kt=2mp
    ([diag | ones]), [:,1,:] masks kt=2mp+1 ([zeros | diag]); the pattern
    is mp-independent. General: [4, 8, 128, 128] per (mp, kt)."""
    allow = ~np.isneginf(np.asarray(mask))  # [q, k] True = allowed
    tiles = []
    for core in range(8):
        par = core % 2

        def ktile_order(kt):
            k = np.arange(128 * kt, 128 * kt + 128)
            if par == 1:
                k = k.reshape(2, 64)[::-1].reshape(128)
            return k

        def qcols(mp):
            # merged block columns = positions 2mp, 2mp+1 -> chunks j=4mp+par, 4mp+2+par
            j0, j1 = 2 * (2 * mp) + par, 2 * (2 * mp + 1) + par
            return np.concatenate([np.arange(64 * j0, 64 * j0 + 64),
                                   np.arange(64 * j1, 64 * j1 + 64)])

        if causal:
            mp = 0
            m = np.zeros((128, 2, 128), dtype=(ml_dtypes.float8_e4m3 if FP8_AV else ml_dtypes.bfloat16))
            q = qcols(mp)
            for i, kt in enumerate((2 * mp, 2 * mp + 1)):
                m[:, i, :] = allow[np.ix_(q, ktile_order(kt))].T
            tiles.append(np.ascontiguousarray(m))
        else:
            m = np.zeros((4, NTT, 128, 128), dtype=(ml_dtypes.float8_e4m3 if FP8_AV else ml_dtypes.bfloat16))
            for mp in range(4):
                q = qcols(mp)
                for kt in range(NTT):
                    m[mp, kt] = allow[np.ix_(q, ktile_order(kt))].T
            tiles.append(m)
    return tiles


def prep_in_maps(inputs):
    bfl = ml_dtypes.bfloat16
    x = np.asarray(inputs['input'], np.float32)      # [B, T, C]
    mask = np.asarray(inputs['mask'], np.float32)
    causal = check_causal(mask)
    Wq = np.asarray(inputs['Wq'], np.float32)        # [H, C, D]
    Wk = np.asarray(inputs['Wk'], np.float32)
    Wv = np.asarray(inputs['Wv'], np.float32)
    Wo = np.asarray(inputs['Wo'], np.float32)        # [C, C]
    w1 = np.asarray(inputs['w1'], np.float32)        # [C, DFF]
    w2 = np.asarray(inputs['w2'], np.float32)
    w3 = np.asarray(inputs['w3'], np.float32)        # [DFF, C]

    f8l = ml_dtypes.float8_e4m3
    qk_dt = f8l if FP8_QK else bfl
    qk_s = S_QK if FP8_QK else 1.0

    def pmaj(a, nct):
        """[X, (ct p), d] -> partition-major [X, p, ct, d] (contiguous per-p
        DMA lines)"""
        x, cpd, dd = a.shape
        return np.ascontiguousarray(
            a.reshape(x, nct, 128, dd).transpose(0, 2, 1, 3))

    wq_l = pmaj((Wq * qk_s).reshape(NP, 2, C, D).transpose(0, 2, 1, 3)
                .reshape(NP, C, 128), NCT).astype(qk_dt)
    wk_l = pmaj((Wk * qk_s).reshape(NP, 2, C, D).transpose(0, 2, 1, 3)
                .reshape(NP, C, 128), NCT).astype(qk_dt)
    wv_l = pmaj((Wv * (S_V if FP8_V else 1.0)).reshape(4, 4, C, D)
                .transpose(0, 2, 1, 3).reshape(4, C, 256),
                NCT).astype(f8l if FP8_V else bfl)
    # wo: [NCT, (half ci p), d] -> [NCT, half, p, ci, d]
    wo_l = np.ascontiguousarray(
        Wo.reshape(C, NCT, 128).transpose(1, 0, 2)
        .reshape(NCT, 2, 4, 128, 128).transpose(0, 1, 3, 2, 4)).astype(bfl)
    w1s = (w1 * S1).reshape(C, 32, 128).transpose(1, 0, 2)  # [32, C, 128]
    if FP8_GATE_FULL:
        w1f_l = pmaj(w1s, NCT).astype(f8l)
        w1a_l = w1b_l = None
    else:
        w1a_l = pmaj(np.ascontiguousarray(w1s[:, :C // 2]),
                     4).astype(f8l if FP8_GATE4 else bfl)
        w1b_l = pmaj(np.ascontiguousarray(w1s[:, C // 2:]), 4).astype(bfl)
    w2_l = pmaj((w2 * (S2 if FP8_VAL else 1.0)).reshape(C, 32, 128)
                .transpose(1, 0, 2), NCT).astype(f8l if FP8_VAL else bfl)
    # w3: [NCT, (dc ft p), d] -> [NCT, dc, p, ft, d]
    w3_l = np.ascontiguousarray(
        (w3 * (S3 if FP8_GVW3 else 1.0)).reshape(DFF, NCT, 128)
        .transpose(1, 0, 2).reshape(NCT, 4, 8, 128, 128)
        .transpose(0, 1, 3, 2, 4)).astype(f8l if FP8_GVW3 else bfl)

    def packp(v):
        return np.ascontiguousarray(np.asarray(v, np.float32).reshape(-1, 128).T)

    cvec_l = np.ascontiguousarray(np.stack(
        [packp(inputs['g1']), packp(inputs['be1']), packp(inputs['g2']),
         packp(inputs['be2']), packp(inputs['bo']), packp(inputs['b3'])],
        axis=1))
    # b2 is added to pvl, which carries the S2 weight scale
    bvec_l = np.ascontiguousarray(np.stack(
        [packp(inputs['b1']),
         packp(np.asarray(inputs['b2'], np.float32) * (S2 if FP8_VAL else 1.0))],
        axis=1))

    mask_tiles = make_mask_tiles(mask, causal)

    in_maps = []
    for core in range(8):
        b, par = core // 2, core % 2
        xt_c = np.ascontiguousarray(x[b].T)            # [C, T]
        if par == 1:
            xt_c = np.ascontiguousarray(swap_cols64(xt_c))
        im = dict(
            xt=xt_c, maskt=mask_tiles[core],
            wq=wq_l, wk=wk_l, wv=wv_l, wo=wo_l, w2=w2_l, w3=w3_l,
            cvec=cvec_l, bvec=bvec_l)
        im['ident'] = np.eye(128, dtype=bfl)
        if FP8_GATE_FULL:
            im['w1f'] = w1f_l
        else:
            im['w1a'] = w1a_l
            im['w1b'] = w1b_l
        in_maps.append(im)
    szflags = dict(
        ln1=bool(np.all(np.asarray(inputs['g1']) == 1)
                 and np.all(np.asarray(inputs['be1']) == 0)),
        ln2=bool(np.all(np.asarray(inputs['g2']) == 1)
                 and np.all(np.asarray(inputs['be2']) == 0)),
        bo=bool(np.all(np.asarray(inputs['bo']) == 0)),
        b1=bool(np.all(np.asarray(inputs['b1']) == 0)),
        b2=bool(np.all(np.asarray(inputs['b2']) == 0)),
        b3=bool(np.all(np.asarray(inputs['b3']) == 0)),
    )
    return in_maps, causal, szflags


def assemble(outs, B=4):
    """outs: list of 8 per-core dicts with 'out' [C, TOK] -> [B, T, C]"""
    full = np.zeros((B, T, C), np.float32)
    for core in range(8):
        b, par = core // 2, core % 2
        o = np.asarray(outs[core]['out']).reshape(C, NCH, 64)
        for p in range(NCH):
            j = 2 * p + par
            full[b, 64 * j:64 * j + 64, :] = o[:, p, :].T
    return full


# ===================== entry point =====================

_NC_CACHE = {}


def _get_nc(causal, sz):
    key = (causal, tuple(sorted(sz.items())))
    if key not in _NC_CACHE:
        _NC_CACHE[key] = build_nc(causal=causal, sz=sz, silu_act=True)
    return _NC_CACHE[key]


def run_on_hw(inputs):
    from concourse import bass2jax
    in_maps, causal, sz = prep_in_maps(inputs)
    nc = _get_nc(causal, sz)
    results = bass2jax.run_bass_via_pjrt(nc, in_maps, n_cores=8)
    return assemble(results)


def kernel(**inputs):
    return run_on_hw(inputs)



# revision 69
# speedup vs baseline: 1.0061x; 1.0009x over previous
"""Self-contained TRN2 kernel for nn_Block_41695542510261 (dense transformer block).

Accepts FULL unsharded inputs, distributes across 8 NeuronCores internally
(2 cores per batch element, causal-balanced 64-row query chunks), returns
the FULL [4, 1024, 1024] output. See build_nc docstring for the design.
"""
import sys, os
for _p in ('/opt/trn_rl_repo', '/root/.axon_site/_ro/trn_rl_repo'):
    if os.path.isdir(_p) and _p not in sys.path:
        sys.path.insert(0, _p)
"""Transformer block kernel for TRN2 — 8-core SPMD, feature-major layout.

Reference: pre-LN attention block + SwiGLU FFN, B=4 T=1024 C=1024 H=16 D=64 DFF=4096.

Sharding: core c handles batch b=c//2, parity par=c%2. Each batch's 16
64-row query chunks split by parity: position p=0..7 <-> chunk j=2p+par.
Causal key-tile count for position p is p+1 for BOTH parities, so one
uniform SPMD program serves all 8 cores. Odd cores receive x with
adjacent 64-column blocks swapped so "own" tokens always sit at even
block positions (compile-time APs stay uniform); key order inside each
128-key tile is permuted consistently for K/V/mask, which attention
sums are invariant to.

Layout: all activations feature-major (xT[c, t]). LN stats via
ones-matmul over the partition (channel) dim + PE outer-product
broadcast. Attention computes S^T = (q.k)^T directly (lhsT=kT, rhs=qT),
softmax without max subtraction (scores bounded; scale 1/32 applied in
the exp), causal masking via 0/1 multiply on the single diagonal key
tile, denominator via a ones-column appended to V, normalization via a
K=1 outer-product matmul (hi/lo split for near-fp32 precision).

Matmuls run in bf16 with fp32 PSUM accumulation, except six fp8
(e4m3, DoubleRow = 2x PE rate) conversions chosen via a numpy
quantization sim validated against measured hw error (sim tracks hw
within ~0.1e-2): Wq/Wk projections, the V projection (its output is
fp8 for AV anyway, so ~free), the FFN val path (w2), gv@w3, P/V in
the attention AV matmul (kt-pair DoubleRow), and half the w1 gate
contraction (full-fp8 gate and fp8 Wo both measured over the 2e-2
gate). Weights are host-scaled by powers of 2 to clear fp8
subnormals; scales divide out in the exp scale / activation scale /
output scale. Weights are also host-repacked partition-major so DMA
partition lines are 1-2KB contiguous (128B lines ran the DMA engine
at ~40GB/s and stalled the w3 phase). Wo's first ci-half runs during
late attention as real filler. The residual path stays fp32.

Reciprocals (LN rsqrt and the softmax denominator) run on the scalar
engine as exp(-ln(x)) / exp(-0.5 ln(x)) (~5e-5 rel, measured): the
DVE RECIPROCAL (1.8-3.3us, free-dim-serial) had been the dominant
PE-stall edge via the score-tile PSUM ring. Ln/Exp/Square share one
act table set; the Silu set is preloaded via a dummy activation
data-pinned after LN2's Exp (a no-dep activation gets hoisted by the
scheduler and forces two extra table swaps).

LN chains write their quantized consumers directly (fast8 path):
bf16-quality mu/rs broadcasts suffice because every consumer is fp8
or bf16-quantized, and the 2-byte vector normalize chain halves the
serial cost. The stats ones-vector carries 1/C so stats matmuls
produce mu / E[x^2] directly.

keep_warm matmuls hold the HAM activity clock at k=8 (k=4 halves the
PE clock; idle quanta trigger it). Warm blocks that should fill a
specific stall are data-pinned (keep_warm_on) to a just-produced
tile; the scheduler hoists dependency-free matmuls away from their
emission point. 256-col warms double LDWEIGHTS overhead (~+27us) —
keep 512-col in hot paths.
"""
import contextlib
import json
import numpy as np
import ml_dtypes

import concourse.bass as bass
import concourse.mybir as mybir
import concourse.tile as tile

f32 = mybir.dt.float32
bf16 = mybir.dt.bfloat16
f8 = mybir.dt.float8e4
AF = mybir.ActivationFunctionType
DR = mybir.MatmulPerfMode.DoubleRow

C = 1024        # d_model
T = 1024        # seq len
H = 16          # heads
D = 64          # head dim
DFF = 4096
TOK = 512       # own tokens per core
NCT = C // 128  # 8 c tiles
NTT = T // 128  # 8 token (key) tiles
NP = H // 2     # 8 head pairs
NCH = 8         # q chunks per core (64 rows each)
EPS = 1e-5

# fp8 (e4m3, DoubleRow 2x matmul) coverage. Error budget per numpy sim:
# qk is ~free (softmax washes it out), val + gvw3 together land ~1.5e-2
# max-rel vs the 2e-2 gate. v/gate/ctxwo stay bf16 (worst err/perf ratio).
FP8_QK = True    # Wq/Wk projections (h, weights fp8)
FP8_VAL = True   # FFN w2 (val) path
FP8_GVW3 = True  # gv -> w3 matmul
FP8_AV = True    # P (exp out), v_all, mask in fp8; AV via DoubleRow kt pairs
FP8_GATE4 = True # FFN w1 (gate): first 4 of 8 ct tiles fp8-DR, rest bf16
FP8_GATE_FULL = False  # FFN w1 fully fp8-DR (hw: 2.096e-2, over gate)
FP8_V = True     # V projection fp8-DR (v_all is fp8 anyway: ~free error)
LN_FAST = True   # bf16 mu/rs LN chains writing fp8/bf16 dst directly
DEN_BUFS = 0     # >0: pden gets its own PSUM pool (sps shrinks to 2)
S1 = 256.0       # host scale on w1 (fp8: clears subnormals; |w1*S1|max ~12 << 448)
S_QK = 64.0      # host scale on Wq/Wk (w~0.02 must clear fp8 subnormals)
S_V = 64.0       # host scale on Wv (fp8)
S2 = 32.0        # host scale on w2 (keeps |gv*S2| < 240; 128 overflows)
S3 = 256.0       # host scale on w3 (fp8 subnormal clearance)
PO_SCALE = 1.0 / (S2 * S3)
SM_SCALE = (1.0 / 32.0) / (S_QK * S_QK if FP8_QK else 1.0)  # 1/sqrt(d_model)


def split_multiwaits(bir_bytes: bytes) -> bytes:
    """Split multi-wait instructions into single-wait EventSemaphore
    carriers placed just before them on the same engine. This walrus
    build has one sync-wait slot for several ISA structs (self-loading
    matmuls, direct DMAs, drains)."""
    m = json.loads(bir_bytes)
    ctr = 0
    for f in m['functions']:
        for blk in f.get('blocks', []):
            insts = blk.get('instructions', [])
            out = []
            changed = False
            for i in insts:
                si = i.get('sync_info')
                w = (si or {}).get('on_wait') or []
                if len(w) > 1:
                    for extra in w[:-1]:
                        ctr += 1
                        out.append({
                            'debug': i.get('debug'),
                            'engine': i['engine'],
                            'ins': [], 'outs': [],
                            'name': f'I-esw-{ctr}',
                            'opcode': 'EventSemaphore',
                            'sync_info': {'on_update': [], 'on_wait': [extra]},
                        })
                    si['on_wait'] = [w[-1]]
                    changed = True
                out.append(i)
            if changed:
                blk['instructions'] = out
    return json.dumps(m).encode()


def patch_nc(nc):
    orig = nc.to_json_bytes
    nc.to_json_bytes = lambda: split_multiwaits(orig())
    return nc


def build_nc(causal=True, sz=None, silu_act=True):
    sz = sz or {}
    nc = bass.Bass(trn_type="TRN2", target_bir_lowering=False, debug=False)

    xt = nc.dram_tensor("xt", [C, T], f32, kind="ExternalInput")
    av_dt = f8 if FP8_AV else bf16
    if causal:
        maskt = nc.dram_tensor("maskt", [128, 2, 128], av_dt, kind="ExternalInput")
    else:
        maskt = nc.dram_tensor("maskt", [4, NTT, 128, 128], av_dt, kind="ExternalInput")
    qk_dt = f8 if FP8_QK else bf16
    w2_dt = f8 if FP8_VAL else bf16
    w3_dt = f8 if FP8_GVW3 else bf16
    # weights arrive pre-transposed to partition-major [.., 128p, ..] so each
    # DMA partition line is 1-2KB contiguous (128B lines ran the DMA engine
    # at ~40GB/s and stalled the w3 phase)
    wq = nc.dram_tensor("wq", [NP, 128, NCT, 128], qk_dt, kind="ExternalInput")
    wk = nc.dram_tensor("wk", [NP, 128, NCT, 128], qk_dt, kind="ExternalInput")
    wv = nc.dram_tensor("wv", [4, 128, NCT, 256], f8 if FP8_V else bf16,
                        kind="ExternalInput")
    wo = nc.dram_tensor("wo", [NCT, 2, 128, 4, 128], bf16, kind="ExternalInput")
    if FP8_GATE_FULL:
        w1f = nc.dram_tensor("w1f", [32, 128, NCT, 128], f8, kind="ExternalInput")
        w1a = w1b = None
    else:
        w1a_dt = f8 if FP8_GATE4 else bf16
        w1a = nc.dram_tensor("w1a", [32, 128, 4, 128], w1a_dt, kind="ExternalInput")
        w1b = nc.dram_tensor("w1b", [32, 128, 4, 128], bf16, kind="ExternalInput")
    w2 = nc.dram_tensor("w2", [32, 128, NCT, 128], w2_dt, kind="ExternalInput")
    w3 = nc.dram_tensor("w3", [NCT, 4, 128, 8, 128], w3_dt, kind="ExternalInput")
    # packed per-channel constants: one DMA instead of eight (each
    # DMA_DIRECT2D costs ~650ns of sync-queue issue time at startup, which
    # delayed the critical xT input fetch)
    cvec = nc.dram_tensor("cvec", [128, 6, NCT], f32, kind="ExternalInput")
    bvec = nc.dram_tensor("bvec", [128, 2, 32], f32, kind="ExternalInput")
    ident = nc.dram_tensor("ident", [128, 128], bf16, kind="ExternalInput")
    out = nc.dram_tensor("out", [C, TOK], f32, kind="ExternalOutput")

    def cnt(p):
        return (p + 1) if causal else NTT

    with tile.TileContext(nc) as tc, contextlib.ExitStack() as ctx:
        consts = ctx.enter_context(tc.tile_pool(name="consts", bufs=1))
        perB = ctx.enter_context(tc.tile_pool(name="perB", bufs=1))
        w12p = ctx.enter_context(tc.tile_pool(name="w12", bufs=6))
        w3p = ctx.enter_context(tc.tile_pool(name="w3_sb", bufs=8))

        # ---- constants ----
        ones_row = consts.tile([1, 128], bf16)
        nc.vector.memset(ones_row, 1.0)
        ones_col = consts.tile([128, 1], bf16)
        # carries the 1/C stats normalization (2^-10, exact in bf16): the
        # stats matmuls then produce mu / E[x^2] directly, removing two
        # serial scalar muls from every LN finish chain
        nc.vector.memset(ones_col, 1.0 / C)
        eps_t = consts.tile([1, 1], f32)
        nc.vector.memset(eps_t, EPS)
        warm_t = consts.tile([128, 512], bf16)
        nc.vector.memset(warm_t, 0.0)
        warm_t8 = consts.tile([128, 128], f8)
        nc.vector.memset(warm_t8, 0.0)
        warm_tf = consts.tile([128, 128], f32)
        nc.vector.memset(warm_tf, 0.0)

        def keep_warm(pool, n, cols=512):
            # dependency-free matmuls that execute during upcoming PE
            # dependency stalls, keeping the HAM clock gate up; smaller
            # cols = finer granularity = less real-work delay
            wp = pool.tile([128, 512], f32, tag="wp")
            for _ in range(n):
                nc.tensor.matmul(wp[:, 0:cols], lhsT=warm_t[:, 0:128],
                                 rhs=warm_t[:, 0:cols], start=True, stop=True)

        def keep_warm_on(pool, n, rhs):
            # dep-pinned warm: the scheduler hoists dependency-free matmuls
            # away from the stall they're meant to fill; reading a
            # just-produced tile anchors them at the right spot
            cols = rhs.free_size()
            wp = pool.tile([128, 512], f32, tag="wp")
            # f32 rhs is NOT supported here: an f32 warm matmul corrupted
            # downstream fp8 results on this hw (PE mode interaction)
            assert rhs.dtype != f32
            lhs = warm_t8 if rhs.dtype == f8 else warm_t
            for _ in range(n):
                nc.tensor.matmul(wp[:, 0:cols], lhsT=lhs[:, 0:128],
                                 rhs=rhs, start=True, stop=True)

        # ---- phase-B persistent tiles (live to the end) ----
        x2T = perB.tile([128, NCT, TOK], f32)
        ln2_fast = LN_FAST and sz.get('ln2', False)
        x2b = None
        if ln2_fast:
            # bf16 cast of x2 (stats side-product) feeds the 2-byte LN2 chain
            x2b = perB.tile([128, NCT, TOK], bf16, tag="x2b")
        h2T = None
        if not FP8_GATE_FULL:
            h2T = perB.tile([128, NCT, TOK], bf16, tag="h2T")
        if FP8_VAL or FP8_GATE4 or FP8_GATE_FULL:
            h2q = perB.tile([128, NCT, TOK], f8)
        else:
            h2q = h2T
        outT = perB.tile([128, NCT, TOK], f32)

        # own-token columns (even 64-blocks) of [:, ct, :]
        def own(tl, ct):
            return tl[:, ct, :].rearrange(
                "p (j two i) -> p j two i", two=2, i=64)[:, :, 0, :]

        # ---------- feature-major layer norm ----------
        def ln_begin(stps, ntok):
            mean_ps = stps.tile([1, ntok], f32, tag="mean")
            ex2_ps = stps.tile([1, ntok], f32, tag="ex2")
            return mean_ps, ex2_ps

        def ln_stats_prep(src_ap, ntok, sqp, xb_dst=None):
            if xb_dst is None:
                xb = sqp.tile([128, ntok], bf16, tag="xb")
            else:
                xb = xb_dst
            nc.vector.tensor_copy(out=xb, in_=src_ap)
            sq = sqp.tile([128, ntok], bf16, tag="sq")
            # square on the scalar engine: vector is the busier queue here
            nc.scalar.activation(sq, xb, AF.Square)
            return xb, sq

        def ln_stats_mm(stats, prep, ct):
            # emitted a couple of cts behind the prep so the in-order PE
            # stream never waits on the vector/scalar prep chain
            mean_ps, ex2_ps = stats
            xb, sq = prep
            nc.tensor.matmul(mean_ps, lhsT=ones_col, rhs=xb,
                             start=(ct == 0), stop=(ct == NCT - 1))
            nc.tensor.matmul(ex2_ps, lhsT=ones_col, rhs=sq,
                             start=(ct == 0), stop=(ct == NCT - 1))

        # src(ct) -> [128, ntok] f32; writes dst(ct) (bf16) or fast8 dst8
        def layer_norm_T(src, dst, ntok, gs, bes, skip_affine,
                         sqp, stps, stss, bcp, tmpp, fast8=None):
            nh = ntok // 512
            assert nh == 1
            stats = ln_begin(stps, ntok)
            preps = []
            for ct in range(NCT):
                xbd = fast8['xb'](ct) if fast8 else None
                preps.append(ln_stats_prep(src(ct), ntok, sqp, xb_dst=xbd))
                if len(preps) >= 3:
                    ct_mm = ct - 2
                    ln_stats_mm(stats, preps[ct_mm], ct_mm)
            for ct_mm in (NCT - 2, NCT - 1):
                ln_stats_mm(stats, preps[ct_mm], ct_mm)
            ln_finish(stats, src, dst, ntok, gs, bes, skip_affine,
                      stss, bcp, tmpp, fast8=fast8)

        def ln_finish(stats, src, dst, ntok, gs, bes, skip_affine,
                      stss, bcp, tmpp, wp_pool=None, n_warm=24, fast8=None):
            mean_ps, ex2_ps = stats
            if fast8 is not None:
                keep_warm_on(wp_pool or bcp, n_warm, fast8['xb'](NCT - 2))
            else:
                keep_warm(wp_pool or bcp, n_warm)
            mu = mean_ps  # ones_col carries 1/C: PSUM rows are mu / E[x^2]
            musq = stss.tile([1, ntok], f32, tag="musq")
            nc.scalar.activation(musq, mean_ps, AF.Square)
            var = stss.tile([1, ntok], f32, tag="var")
            nc.vector.tensor_sub(var, ex2_ps, musq)
            # rs = exp(-0.5*ln(var+eps)) on the scalar engine: ~3e-5 rel
            # (measured), replaces Sqrt + the 3.3us single-partition DVE
            # reciprocal on the critical LN chain
            lnv = stss.tile([1, ntok], f32, tag="lnv")
            nc.scalar.activation(lnv, var, AF.Ln, bias=eps_t)
            rs = stss.tile([1, ntok], f32, tag="rs")
            nc.scalar.activation(rs, lnv, AF.Exp, scale=-0.5)
            mu_hi = stss.tile([1, ntok], bf16, tag="mu_hi")
            nc.vector.tensor_copy(out=mu_hi, in_=mu)
            if fast8 is None:
                rs_hi = stss.tile([1, ntok], bf16, tag="rs_hi")
                nc.vector.tensor_copy(out=rs_hi, in_=rs)
            if fast8 is not None:
                # consumers are fp8 (or bf16-quantized anyway): bf16-quality
                # mu/rs suffice, so use single-pass broadcasts and a 2-byte
                # vector normalize chain writing the quantized dst directly
                # (halves the serial chain and removes the cast trail that
                # gated the first downstream matmul)
                assert skip_affine
                mu_bc = bcp.tile([128, ntok], f32, tag="mu_bc")
                rs_bc = bcp.tile([128, ntok], f32, tag="rs_bc")
                nc.tensor.matmul(mu_bc, lhsT=ones_row, rhs=mu_hi,
                                 start=True, stop=True)
                # redundant mu-broadcasts bridge the PE over the Ln/Exp wait
                wpb = (wp_pool or bcp).tile([128, 512], f32, tag="wp")
                for _ in range(2):
                    nc.tensor.matmul(wpb[:, 0:ntok], lhsT=ones_row, rhs=mu_hi,
                                     start=True, stop=True)
                rs_hi = stss.tile([1, ntok], bf16, tag="rs_hi")
                nc.vector.tensor_copy(out=rs_hi, in_=rs)
                nc.tensor.matmul(rs_bc, lhsT=ones_row, rhs=rs_hi,
                                 start=True, stop=True)
                mu_sb = stss.tile([128, ntok], bf16, tag="mu_sb")
                nc.scalar.copy(out=mu_sb, in_=mu_bc)
                rs_sb = stss.tile([128, ntok], bf16, tag="rs_sb")
                nc.scalar.copy(out=rs_sb, in_=rs_bc)
                pw_n, pw_cols = fast8.get('post_warm', (12, 256))
                keep_warm_on(wp_pool or bcp, pw_n, mu_sb[:, 0:pw_cols])
                if fast8.get('preload_silu'):
                    # pull the Silu table swap off the first FFN activation:
                    # load it now, hidden behind the vector normalize chain.
                    # input dep on rs pins it AFTER the Ln/Exp pair (a no-dep
                    # activation gets hoisted by the scheduler and forces two
                    # extra table swaps)
                    dummy = stss.tile([1, 1], f32, tag="dummy")
                    nc.scalar.activation(dummy, rs[0:1, 0:1], AF.Silu)
                for ct in range(NCT):
                    # fp8-out mul costs 1.37us vs 0.41us bf16 (fast DVE mode
                    # lost), but skipping the bf16 intermediate is worth
                    # ~0.24e-2 of the error budget (double rounding)
                    tmpb = tmpp.tile([128, ntok], bf16, tag="lntmpb")
                    nc.vector.tensor_sub(tmpb, fast8['xb'](ct), mu_sb)
                    nc.vector.tensor_mul(fast8['dst8'](ct), tmpb, rs_sb)
                return
            # hi/lo split of mu and rs for near-fp32 broadcast
            mu_lob = stss.tile([1, ntok], bf16, tag="mu_lob")
            nc.vector.tensor_sub(mu_lob, mu, mu_hi)
            rs_lob = stss.tile([1, ntok], bf16, tag="rs_lob")
            nc.vector.tensor_sub(rs_lob, rs, rs_hi)
            mu_bc = bcp.tile([128, ntok], f32, tag="mu_bc")
            rs_bc = bcp.tile([128, ntok], f32, tag="rs_bc")
            nc.tensor.matmul(mu_bc, lhsT=ones_row, rhs=mu_hi,
                             start=True, stop=False)
            nc.tensor.matmul(mu_bc, lhsT=ones_row, rhs=mu_lob,
                             start=False, stop=True)
            nc.tensor.matmul(rs_bc, lhsT=ones_row, rhs=rs_hi,
                             start=True, stop=False)
            nc.tensor.matmul(rs_bc, lhsT=ones_row, rhs=rs_lob,
                             start=False, stop=True)
            for ct in range(NCT):
                tmp = tmpp.tile([128, ntok], f32, tag="lntmp")
                nc.vector.tensor_sub(tmp, src(ct), mu_bc)
                if skip_affine:
                    nc.vector.tensor_mul(dst(ct), tmp, rs_bc)
                else:
                    nc.vector.tensor_mul(tmp, tmp, rs_bc)
                    nc.scalar.activation(dst(ct), tmp, AF.Identity,
                                         bias=bes[:, ct:ct + 1],
                                         scale=gs[:, ct:ct + 1])

        with tc.tile_pool(name="perA", bufs=1) as perA:
            # ---- phase-A persistent tiles ----
            xT = perA.tile([128, NCT, T], f32)
            if LN_FAST and FP8_QK and FP8_V and sz.get('ln1', False):
                # h exists only as fp8; xbT (bf16 cast of x, stats
                # side-product) feeds the 2-byte normalize chain
                xbT = perA.tile([128, NCT, T], bf16)
                hT = None
                hq = perA.tile([128, NCT, T], f8)
            else:
                xbT = None
                hT = perA.tile([128, NCT, T], bf16)
                if FP8_QK:
                    hq = perA.tile([128, NCT, T], f8)
                else:
                    hq = hT
            v_all = perA.tile([128, NTT, H, 65], av_dt)
            ctxT = perA.tile([128, NCT, TOK], bf16)
            wo_all = perA.tile([128, NCT, 4, 128], bf16)
            x2h1 = perA.tile([128, NCT, TOK], bf16)

            for th in range(2):
                for ct in range(NCT):
                    nc.sync.dma_start(
                        out=xT[:, ct, th * 512:(th + 1) * 512],
                        in_=xt[ct * 128:(ct + 1) * 128, th * 512:(th + 1) * 512])
            # constants issued after the critical xT input stream
            cv = consts.tile([128, 6, NCT], f32)
            nc.sync.dma_start(out=cv, in_=cvec[:, :, :])
            g1s, be1s, g2s, be2s, bos, b3s = (cv[:, i, :] for i in range(6))
            bv = consts.tile([128, 2, 32], f32)
            nc.sync.dma_start(out=bv, in_=bvec[:, :, :])
            id_sb = consts.tile([128, 128], bf16)
            nc.sync.dma_start(out=id_sb, in_=ident[:, :])
            b1s = bv[:, 0, :]
            b2s = bv[:, 1, :]
            if causal:
                mk = consts.tile([128, 2, 128], av_dt)
                nc.sync.dma_start(out=mk, in_=maskt[:, :, :])
            else:
                mk = consts.tile([128, 4, NTT, 128], av_dt)
                nc.sync.dma_start(
                    out=mk, in_=maskt[:, :, :, :].rearrange("c k p q -> p c k q"))

            with tc.tile_pool(name="ln_sq", bufs=3) as sqp, \
                 tc.tile_pool(name="ln_st", bufs=1, space="PSUM") as stps, \
                 tc.tile_pool(name="ln_sts", bufs=1) as stss, \
                 tc.tile_pool(name="ln_bc", bufs=1, space="PSUM") as bcp, \
                 tc.tile_pool(name="ln_tmp", bufs=2) as tmpp, \
                 tc.tile_pool(name="wv_sb", bufs=1) as wvp, \
                 tc.tile_pool(name="v_ps", bufs=2, space="PSUM") as vps:
                # prefetch both V weight halves up front
                wv_dt = f8 if FP8_V else bf16
                wv_sbs = []
                for g in range(2):
                    wv_sb = wvp.tile([128, NCT, 2, 256], wv_dt, tag=f"wv{g}")
                    for q in range(2):
                        nc.sync.dma_start(
                            out=wv_sb[:, :, q, :], in_=wv[2 * g + q])
                    wv_sbs.append(wv_sb)

                def v_block(tts):
                    for g in range(2):
                        for tt in tts:
                            pv = vps.tile([128, 512], f32, tag="pv")
                            if FP8_V:
                                for g2 in range(4):
                                    nc.tensor.matmul(
                                        pv,
                                        lhsT=hq[:, 2 * g2:2 * g2 + 2,
                                                tt * 128:(tt + 1) * 128],
                                        rhs=wv_sbs[g][:, 2 * g2:2 * g2 + 2, :, :],
                                        perf_mode=DR,
                                        start=(g2 == 0), stop=(g2 == 3))
                                nc.scalar.activation(
                                    v_all[:, tt, 8 * g:8 * (g + 1), 0:64],
                                    pv[:, :].rearrange("p (h d) -> p h d", d=64),
                                    AF.Copy, scale=1.0 / S_V)
                            else:
                                for ct in range(NCT):
                                    nc.tensor.matmul(
                                        pv, lhsT=hT[:, ct, tt * 128:(tt + 1) * 128],
                                        rhs=wv_sbs[g][:, ct, :, :],
                                        start=(ct == 0), stop=(ct == NCT - 1))
                                nc.scalar.copy(
                                    out=v_all[:, tt, 8 * g:8 * (g + 1), 0:64],
                                    in_=pv[:, :].rearrange("p (h d) -> p h d", d=64))

                # LN half 0 -> V for its token tiles fills LN half 1's
                # dependency stall with real matmuls; then LN half 1 -> rest
                for th in range(2):
                    tsl = slice(th * 512, (th + 1) * 512)
                    if xbT is not None:
                        layer_norm_T(lambda ct: xT[:, ct, tsl], None, 512,
                                     g1s, be1s, sz.get('ln1', False),
                                     sqp, stps, stss, bcp, tmpp,
                                     fast8=dict(
                                         xb=lambda ct: xbT[:, ct, tsl],
                                         dst8=lambda ct: hq[:, ct, tsl],
                                         post_warm=(10, 256)))
                    else:
                        layer_norm_T(lambda ct: xT[:, ct, tsl],
                                     lambda ct: hT[:, ct, tsl], 512,
                                     g1s, be1s, sz.get('ln1', False),
                                     sqp, stps, stss, bcp, tmpp)
                    if FP8_QK and xbT is None and FP8_V:
                        # fp8 V consumes hq: cast before v_block
                        for ct in range(NCT):
                            nc.scalar.copy(out=hq[:, ct, tsl],
                                           in_=hT[:, ct, tsl])
                    v_block(range(4 * th, 4 * th + 4))
                    # after v_block so the casts don't block V evacuation
                    # on the in-order scalar queue
                    if FP8_QK and xbT is None and not FP8_V:
                        for ct in range(NCT):
                            nc.scalar.copy(out=hq[:, ct, tsl],
                                           in_=hT[:, ct, tsl])
            nc.vector.memset(v_all[:, :, :, 64:65], 1.0)
            for cot in range(NCT):
                nc.sync.dma_start(
                    out=wo_all[:, cot, :, :],
                    in_=wo[cot, 0])

            # ---------- attention ----------
            with tc.tile_pool(name="wqk", bufs=2) as wqkp, \
                 tc.tile_pool(name="qk_ps", bufs=1, space="PSUM") as qkps, \
                 tc.tile_pool(name="qk_sb", bufs=2) as qksb, \
                 tc.tile_pool(name="s_ps", bufs=(2 if DEN_BUFS else 3),
                              space="PSUM") as sps, \
                 tc.tile_pool(name="p_sb", bufs=2) as psb, \
                 tc.tile_pool(name="ctx_ps", bufs=3, space="PSUM") as cps, \
                 tc.tile_pool(name="at_wp", bufs=1, space="PSUM") as wps, \
                 tc.tile_pool(name="nrm_sb", bufs=2) as nsb, \
                 contextlib.ExitStack() as dctx:
                dps = dctx.enter_context(
                    tc.tile_pool(name="den_ps", bufs=DEN_BUFS, space="PSUM")) \
                    if DEN_BUFS else None
                def emit_scores_both(qT, kT, P0, P1, m):
                    # merged mp pair {2m, 2m+1}: one 256-wide q block per
                    # matmul — halves the small-matmul count (the attention
                    # phase is per-instruction-overhead bound, ~150ns fixed
                    # cost on a 107ns stream). h2=0 rows 0:64, h2=1 rows
                    # 64:128 co-execute via PE row packing.
                    n_kt = cnt(4 * m + 3)
                    qsl = slice(m * 256, (m + 1) * 256)
                    for kg in range(0, n_kt, 2):
                        # the block at kg==4m+2 is causally dead for sub-mp
                        # 2m (cols 0:128): compute/exp only the valid half
                        # and memset the dead half (sole writer: no cross-
                        # engine WAW race, no serialization)
                        dead = causal and kg == 4 * m + 2
                        csl = slice(128, 256) if dead else slice(0, 256)
                        qs2 = slice(qsl.start + csl.start, qsl.start + csl.stop)
                        ps0 = sps.tile([128, 2, 256], f32, tag="ps_s")
                        ps1 = sps.tile([128, 2, 256], f32, tag="ps_s")
                        for kt in range(kg, kg + 2):
                            nc.tensor.matmul(
                                ps0[:, kt - kg, csl],
                                lhsT=kT[0:64, kt * 128:(kt + 1) * 128],
                                rhs=qT[0:64, qs2], start=True, stop=True)
                            nc.tensor.matmul(
                                ps1[:, kt - kg, csl],
                                lhsT=kT[64:128, kt * 128:(kt + 1) * 128],
                                rhs=qT[64:128, qs2], start=True, stop=True)
                        nc.scalar.activation(P0[:, kg:kg + 2, csl],
                                             ps0[:, :, csl],
                                             AF.Exp, scale=SM_SCALE)
                        nc.scalar.activation(P1[:, kg:kg + 2, csl],
                                             ps1[:, :, csl],
                                             AF.Exp, scale=SM_SCALE)
                        if dead:
                            nc.vector.memset(P0[:, kg:kg + 2, 0:128], 0.0)
                            nc.vector.memset(P1[:, kg:kg + 2, 0:128], 0.0)
                    for P in (P0, P1):
                        if causal:
                            # diag masks: sub-mp 2m on cols 0:128 (kts
                            # 4m..4m+1), sub-mp 2m+1 on cols 128:256
                            nc.vector.tensor_mul(
                                P[:, 4 * m:4 * m + 2, 0:128],
                                P[:, 4 * m:4 * m + 2, 0:128], mk)
                            nc.vector.tensor_mul(
                                P[:, 4 * m + 2:4 * m + 4, 128:256],
                                P[:, 4 * m + 2:4 * m + 4, 128:256], mk)

                        else:
                            nc.vector.tensor_mul(P[:, 0:n_kt, 0:128],
                                                 P[:, 0:n_kt, 0:128],
                                                 mk[:, 2 * m, 0:n_kt, :])
                            nc.vector.tensor_mul(P[:, 0:n_kt, 128:256],
                                                 P[:, 0:n_kt, 128:256],
                                                 mk[:, 2 * m + 1, 0:n_kt, :])

                def emit_av(P, pctx, hp, m, h2):
                    n_kt = cnt(4 * m + 3)
                    h = 2 * hp + h2
                    if FP8_AV and n_kt % 2 == 0:
                        # DoubleRow over kt pairs: halves AV matmul count
                        npair = n_kt // 2
                        for j in range(npair):
                            nc.tensor.matmul(
                                pctx[:, h2, :],
                                lhsT=v_all[:, 2 * j:2 * j + 2, h, :],
                                rhs=P[:, 2 * j:2 * j + 2, :], perf_mode=DR,
                                start=(j == 0), stop=(j == npair - 1))
                    else:
                        for kt in range(n_kt):
                            nc.tensor.matmul(
                                pctx[:, h2, :], lhsT=v_all[:, kt, h, :],
                                rhs=P[:, kt, :],
                                start=(kt == 0), stop=(kt == n_kt - 1))

                def emit_den(pctx):
                    # hi/lo bf16 split of the softmax denominator row so the
                    # PE ones-broadcast reconstructs it at ~fp32 in PSUM
                    d_hi = nsb.tile([1, 512], bf16, tag="d_hi")
                    nc.vector.tensor_copy(out=d_hi, in_=pctx[64:65, :, :])
                    d_lo = nsb.tile([1, 512], bf16, tag="d_lo")
                    nc.vector.tensor_sub(d_lo, pctx[64:65, :, :], d_hi)
                    return d_hi, d_lo

                def emit_norm_pair(ga, gb):
                    # normalize two merged groups (= 4 original mp groups)
                    # with ONE 1/den chain: A on partitions 0:64, B on 64:128
                    if DEN_BUFS:
                        pden = dps.tile([128, 512], f32, tag="pden")
                    else:
                        pden = sps.tile([128, 512], f32, tag="ps_s")
                    for row, g in ((0, ga), (64, gb)):
                        if g is None:
                            continue
                        _, d_hi, d_lo, _, _ = g
                        nc.tensor.matmul(pden[row:row + 64, :],
                                         lhsT=ones_row[0:1, 0:64], rhs=d_hi,
                                         start=True, stop=False)
                        nc.tensor.matmul(pden[row:row + 64, :],
                                         lhsT=ones_row[0:1, 0:64], rhs=d_lo,
                                         start=False, stop=True)
                    # 1/den = exp(-ln(den)) on the scalar engine (~5e-5 rel,
                    # measured): frees the pden ring after the quick Ln and
                    # keeps the DVE RECIPROCAL off the busy vector queue
                    lnd = nsb.tile([128, 512], f32, tag="lnd")
                    nrm2 = nsb.tile([128, 512], f32, tag="nrm2")
                    if gb is None:
                        nc.scalar.activation(lnd[0:64, :], pden[0:64, :], AF.Ln)
                        nc.scalar.activation(nrm2[0:64, :], lnd[0:64, :],
                                             AF.Exp, scale=-1.0)
                    else:
                        nc.scalar.activation(lnd, pden, AF.Ln)
                        nc.scalar.activation(nrm2, lnd, AF.Exp, scale=-1.0)
                    for row, g in ((0, ga), (64, gb)):
                        if g is None:
                            continue
                        pctx, _, _, php, pm = g
                        for h2 in range(2):
                            nc.vector.tensor_mul(
                                ctxT[64 * h2:64 * (h2 + 1), php,
                                     pm * 256:(pm + 1) * 256],
                                pctx[0:64, h2, :],
                                nrm2[row:row + 64, 256 * h2:256 * (h2 + 1)])

                # pipeline over merged groups: scores(g+1) are emitted before
                # av(g) so the in-order PE stream never drains while the
                # exp/mask chain runs; norm/evac for g trails one group.
                # qk projection emission, split so pair hp+1's dense
                # N=512 matmuls can be interleaved into pair hp's attention
                # groups (fills exp/mask bubbles in the in-order PE stream)
                def own2(tl, g):
                    # own-token (even 64-block) columns for ct pair 2g, 2g+1
                    return tl[:, 2 * g:2 * g + 2, :].rearrange(
                        "p c (j two i) -> p c j two i", two=2, i=64)[:, :, :, 0, :]

                def make_qk_parts(hp):
                    wq_sb = wqkp.tile([128, NCT, 128], qk_dt, tag="wq")
                    nc.sync.dma_start(
                        out=wq_sb, in_=wq[hp])
                    wk_sb = wqkp.tile([128, NCT, 128], qk_dt, tag="wk")
                    nc.sync.dma_start(
                        out=wk_sb, in_=wk[hp])
                    qT = qksb.tile([128, 512], bf16, tag="qT")
                    kT = qksb.tile([128, 1024], bf16, tag="kT")

                    def part_q():
                        pq = qkps.tile([128, 512], f32, tag="pqk")
                        if FP8_QK:
                            for g in range(4):
                                nc.tensor.matmul(pq, lhsT=wq_sb[:, 2 * g:2 * g + 2, :],
                                                 rhs=own2(hq, g), perf_mode=DR,
                                                 start=(g == 0), stop=(g == 3))
                        else:
                            for ct in range(NCT):
                                nc.tensor.matmul(pq, lhsT=wq_sb[:, ct, :],
                                                 rhs=own(hT, ct),
                                                 start=(ct == 0), stop=(ct == NCT - 1))
                        nc.scalar.copy(out=qT, in_=pq)

                    def part_k(hh):
                        sl = slice(hh * 512, (hh + 1) * 512)
                        pk = qkps.tile([128, 512], f32, tag="pqk")
                        if FP8_QK:
                            for g in range(4):
                                nc.tensor.matmul(pk, lhsT=wk_sb[:, 2 * g:2 * g + 2, :],
                                                 rhs=hq[:, 2 * g:2 * g + 2, sl],
                                                 perf_mode=DR,
                                                 start=(g == 0), stop=(g == 3))
                        else:
                            for ct in range(NCT):
                                nc.tensor.matmul(pk, lhsT=wk_sb[:, ct, :],
                                                 rhs=hT[:, ct, sl],
                                                 start=(ct == 0), stop=(ct == NCT - 1))
                        nc.scalar.copy(out=kT[:, sl], in_=pk)

                    return qT, kT, (part_q, lambda: part_k(0), lambda: part_k(1))

                prev = None          # (P0, P1, hp, m) awaiting av
                pend = []            # groups awaiting a paired norm
                wo_early = list(range(NCT))  # Wo ci 0..3 half, run as filler
                qT, kT, parts = make_qk_parts(0)
                for pf in parts:
                    pf()
                nxt = None
                for hp in range(NP):
                    if hp + 1 < NP:
                        nxt = make_qk_parts(hp + 1)
                    for m in range(2):
                        P0 = psb.tile([128, NTT, 256], av_dt, tag="P0")
                        P1 = psb.tile([128, NTT, 256], av_dt, tag="P1")
                        emit_scores_both(qT, kT, P0, P1, m)
                        if hp >= 5 and wo_early:
                            cot = wo_early.pop(0)
                            pa1 = wps.tile([128, TOK], f32, tag="wp")
                            for ci in range(4):
                                nc.tensor.matmul(pa1, lhsT=wo_all[:, cot, ci, :],
                                                 rhs=ctxT[:, ci, :],
                                                 start=(ci == 0), stop=(ci == 3))
                            nc.scalar.copy(out=x2h1[:, cot, :], in_=pa1)
                            keep_warm_on(wps, 2, P0[:, 0:2, :])
                        else:
                            keep_warm(wps, 4)
                            keep_warm_on(wps, 3, P0[:, 0:2, :])
                        if hp + 1 < NP:
                            if m == 0:
                                nxt[2][0]()  # pair hp+1 q projection filler
                            else:
                                nxt[2][1]()  # pair hp+1 k halves
                                nxt[2][2]()
                        if len(pend) == 2:
                            emit_norm_pair(pend[0], pend[1])
                            pend = []
                        if prev is not None:
                            pP0, pP1, php, pm = prev
                            pctx = cps.tile([65, 2, 256], f32, tag="pctx")
                            emit_av(pP0, pctx, php, pm, 0)
                            emit_av(pP1, pctx, php, pm, 1)
                            d_hi, d_lo = emit_den(pctx)
                            pend.append((pctx, d_hi, d_lo, php, pm))
                        prev = (P0, P1, hp, m)
                    if hp + 1 < NP:
                        qT, kT, _ = nxt[0], nxt[1], None
                # drain
                if prev is not None:
                    pP0, pP1, php, pm = prev
                    pctx = cps.tile([65, 2, 256], f32, tag="pctx")
                    emit_av(pP0, pctx, php, pm, 0)
                    emit_av(pP1, pctx, php, pm, 1)
                    d_hi, d_lo = emit_den(pctx)
                    pend.append((pctx, d_hi, d_lo, php, pm))
                while pend:
                    ga = pend.pop(0)
                    gb = pend.pop(0) if pend else None
                    emit_norm_pair(ga, gb)
                    keep_warm(wps, 4)
                while wo_early:
                    cot = wo_early.pop(0)
                    pa1 = wps.tile([128, TOK], f32, tag="wp")
                    for ci in range(4):
                        nc.tensor.matmul(pa1, lhsT=wo_all[:, cot, ci, :],
                                         rhs=ctxT[:, ci, :],
                                         start=(ci == 0), stop=(ci == 3))
                    nc.scalar.copy(out=x2h1[:, cot, :], in_=pa1)

            # ---------- Wo + residual, LN2 stats interleaved per cot ----------
            with tc.tile_pool(name="wo_sb", bufs=2) as wop, \
                 tc.tile_pool(name="a_ps", bufs=2, space="PSUM") as aps, \
                 tc.tile_pool(name="a_sb", bufs=2) as asb, \
                 tc.tile_pool(name="l2_sq", bufs=3) as sqp2, \
                 tc.tile_pool(name="l2_st", bufs=1, space="PSUM") as stps2, \
                 tc.tile_pool(name="l2_sts", bufs=1) as stss2, \
                 tc.tile_pool(name="l2_bc", bufs=1, space="PSUM") as bcp2, \
                 tc.tile_pool(name="l2_tmp", bufs=2) as tmpp2:
                keep_warm(aps, 6)
                stats2 = ln_begin(stps2, TOK)
                preps2 = []
                for cot in range(NCT):
                    wo_sb = wop.tile([128, 4, 128], bf16, tag="wo")
                    nc.sync.dma_start(
                        out=wo_sb,
                        in_=wo[cot, 1])
                    pa = aps.tile([128, TOK], f32, tag="pa")
                    for ci in range(4):
                        nc.tensor.matmul(pa, lhsT=wo_sb[:, ci, :],
                                         rhs=ctxT[:, 4 + ci, :],
                                         start=(ci == 0), stop=False)
                    # fold the x2h1 (Wo first-half partial) add into the
                    # PSUM via identity weights: +0.43us on the starving PE
                    # here buys back a 1.37us f32 vector add per cot
                    nc.tensor.matmul(pa, lhsT=id_sb, rhs=x2h1[:, cot, :],
                                     start=False, stop=True)
                    if sz.get('bo', False):
                        nc.vector.tensor_add(x2T[:, cot, :], pa, own(xT, cot))
                    else:
                        tmpa = asb.tile([128, TOK], f32, tag="tmpa")
                        nc.scalar.activation(tmpa, pa, AF.Identity,
                                             bias=bos[:, cot:cot + 1], scale=1.0)
                        nc.vector.tensor_add(x2T[:, cot, :], tmpa, own(xT, cot))
                    preps2.append(ln_stats_prep(
                        x2T[:, cot, :], TOK, sqp2,
                        xb_dst=(x2b[:, cot, :] if ln2_fast else None)))
                    if len(preps2) >= 3:
                        ln_stats_mm(stats2, preps2[cot - 2], cot - 2)
                for ct_mm in (NCT - 2, NCT - 1):
                    ln_stats_mm(stats2, preps2[ct_mm], ct_mm)
                if ln2_fast:
                    n8 = NCT if FP8_GATE_FULL else 4

                    def ln2_dst8(ct):
                        return h2q[:, ct, :] if ct < n8 else h2T[:, ct, :]
                    ln_finish(stats2, lambda ct: x2T[:, ct, :], None, TOK,
                              g2s, be2s, True, stss2, bcp2, tmpp2,
                              wp_pool=aps, n_warm=10,
                              fast8=dict(xb=lambda ct: x2b[:, ct, :],
                                         dst8=ln2_dst8,
                                         post_warm=(12, 256),
                                         preload_silu=True))
                    if FP8_VAL and not FP8_GATE_FULL:
                        for ct in range(n8, NCT):
                            nc.scalar.copy(out=h2q[:, ct, :], in_=h2T[:, ct, :])
                else:
                    ln_finish(stats2, lambda ct: x2T[:, ct, :],
                              lambda ct: h2T[:, ct, :], TOK, g2s, be2s,
                              sz.get('ln2', False), stss2, bcp2, tmpp2,
                              wp_pool=aps, n_warm=10)
                    if FP8_VAL or FP8_GATE_FULL:
                        for ct in range(NCT):
                            nc.scalar.copy(out=h2q[:, ct, :], in_=h2T[:, ct, :])

        # ---------- FFN ----------
        with tc.tile_pool(name="g_ps", bufs=2, space="PSUM") as gps, \
             tc.tile_pool(name="vl_ps", bufs=2, space="PSUM") as vlps, \
             tc.tile_pool(name="g_sb", bufs=2) as gsbp, \
             tc.tile_pool(name="gv_sb", bufs=1) as gvp, \
             tc.tile_pool(name="o_ps", bufs=2, space="PSUM") as ops:
            gv_all = gvp.tile([128, 4, 8, TOK], w3_dt, tag="gv")
            # lookahead w3 weight fetch: issue DMAs well before the w3 loop
            # so its matmuls never wait on HBM
            w3_tiles = {}
            w3_next = [0]

            def w3_fetch_upto(n):
                while w3_next[0] < min(n, 4 * NCT):
                    i = w3_next[0]
                    cot, dc = divmod(i, 4)
                    t3 = w3p.tile([128, 8, 128], w3_dt, tag="w3")
                    nc.sync.dma_start(
                        out=t3,
                        in_=w3[cot, dc])
                    w3_tiles[i] = t3
                    w3_next[0] += 1

            for dc in range(4):
                gv = gv_all[:, dc, :, :]
                for fi in range(8):
                    ft = dc * 8 + fi
                    if dc == 3:
                        w3_fetch_upto(fi)
                    if FP8_GATE_FULL:
                        w1_sb = w12p.tile([128, NCT, 128], f8, tag="w1f")
                        nc.sync.dma_start(
                            out=w1_sb,
                            in_=w1f[ft])
                    else:
                        w1a_sb = w12p.tile([128, 4, 128], w1a_dt, tag="w1a")
                        nc.sync.dma_start(
                            out=w1a_sb, in_=w1a[ft])
                        w1b_sb = w12p.tile([128, 4, 128], bf16, tag="w1b")
                        nc.sync.dma_start(
                            out=w1b_sb, in_=w1b[ft])
                    w2_sb = w12p.tile([128, NCT, 128], w2_dt, tag="w2")
                    nc.sync.dma_start(
                        out=w2_sb, in_=w2[ft])
                    pg = gps.tile([128, TOK], f32, tag="pg")
                    pvl = vlps.tile([128, TOK], f32, tag="pvl")
                    if FP8_GATE_FULL:
                        for g in range(4):
                            nc.tensor.matmul(pg, lhsT=w1_sb[:, 2 * g:2 * g + 2, :],
                                             rhs=h2q[:, 2 * g:2 * g + 2, :],
                                             perf_mode=DR,
                                             start=(g == 0), stop=(g == 3))
                    elif FP8_GATE4:
                        for g in range(2):
                            nc.tensor.matmul(pg, lhsT=w1a_sb[:, 2 * g:2 * g + 2, :],
                                             rhs=h2q[:, 2 * g:2 * g + 2, :],
                                             perf_mode=DR,
                                             start=(g == 0), stop=False)
                    else:
                        for ci in range(4):
                            nc.tensor.matmul(pg, lhsT=w1a_sb[:, ci, :],
                                             rhs=h2T[:, ci, :],
                                             start=(ci == 0), stop=False)
                    if not FP8_GATE_FULL:
                        for ci in range(4):
                            nc.tensor.matmul(pg, lhsT=w1b_sb[:, ci, :],
                                             rhs=h2T[:, 4 + ci, :],
                                             start=False, stop=(ci == 3))
                    if FP8_VAL:
                        for g in range(4):
                            nc.tensor.matmul(pvl, lhsT=w2_sb[:, 2 * g:2 * g + 2, :],
                                             rhs=h2q[:, 2 * g:2 * g + 2, :],
                                             perf_mode=DR,
                                             start=(g == 0), stop=(g == 3))
                    else:
                        for ct in range(NCT):
                            nc.tensor.matmul(pvl, lhsT=w2_sb[:, ct, :],
                                             rhs=h2T[:, ct, :],
                                             start=(ct == 0), stop=(ct == NCT - 1))
                    gs_t = gsbp.tile([128, TOK], f32, tag="gs_t")
                    if silu_act:
                        nc.scalar.activation(gs_t, pg, AF.Silu,
                                             bias=b1s[:, ft:ft + 1], scale=1.0 / S1)
                    else:
                        # silu(x) = x * sigmoid(x); x = pg + b1
                        nc.scalar.activation(gs_t, pg, AF.Sigmoid,
                                             bias=b1s[:, ft:ft + 1], scale=1.0 / S1)
                        if sz.get('b1', False):
                            nc.vector.tensor_mul(gs_t, gs_t, pg)
                        else:
                            xg = gsbp.tile([128, TOK], f32, tag="xg")
                            nc.vector.tensor_scalar_add(xg, pg, b1s[:, ft:ft + 1])
                            nc.vector.tensor_mul(gs_t, gs_t, xg)
                    if sz.get('b2', False):
                        nc.vector.tensor_mul(gv[:, fi, :], pvl, gs_t)
                    else:
                        nc.vector.tensor_scalar_add(gv[:, fi, :], pvl,
                                                    b2s[:, ft:ft + 1])
                        nc.vector.tensor_mul(gv[:, fi, :], gv[:, fi, :], gs_t)
            # cot-major w3: all 4 dc chunks accumulate in one PSUM group,
            # one scale+add per output tile (replaces 32 vector accumulates)
            po_s = (S2 if FP8_VAL else 1.0) * (S3 if FP8_GVW3 else 1.0)
            for cot in range(NCT):
                po = ops.tile([128, TOK], f32, tag="po")
                for dc in range(4):
                    w3_fetch_upto(4 * cot + dc + 7)
                    w3_sb = w3_tiles.pop(4 * cot + dc)
                    if FP8_GVW3:
                        for g in range(4):
                            nc.tensor.matmul(po, lhsT=w3_sb[:, 2 * g:2 * g + 2, :],
                                             rhs=gv_all[:, dc, 2 * g:2 * g + 2, :],
                                             perf_mode=DR,
                                             start=(dc == 0 and g == 0),
                                             stop=(dc == 3 and g == 3))
                    else:
                        for fi in range(8):
                            nc.tensor.matmul(po, lhsT=w3_sb[:, fi, :],
                                             rhs=gv_all[:, dc, fi, :],
                                             start=(dc == 0 and fi == 0),
                                             stop=(dc == 3 and fi == 7))
                if not sz.get('b3', False):
                    tmpo = gsbp.tile([128, TOK], f32, tag="tmpo")
                    nc.scalar.activation(tmpo, po, AF.Identity,
                                         bias=b3s[:, cot:cot + 1],
                                         scale=1.0 / po_s)
                    nc.vector.tensor_add(outT[:, cot, :], tmpo, x2T[:, cot, :])
                else:
                    nc.vector.scalar_tensor_tensor(
                        out=outT[:, cot, :], in0=po, scalar=1.0 / po_s,
                        in1=x2T[:, cot, :], op0=mybir.AluOpType.mult,
                        op1=mybir.AluOpType.add)
                nc.sync.dma_start(out=out[cot * 128:(cot + 1) * 128, :],
                                  in_=outT[:, cot, :])
    patch_nc(nc)
    return nc


# ===================== host-side prep =====================

def swap_cols64(a):
    """swap adjacent 64-col blocks along last axis"""
    s = a.shape
    b = a.reshape(*s[:-1], s[-1] // 128, 2, 64)
    return b[..., ::-1, :].reshape(s)


def check_causal(mask):
    T_ = mask.shape[0]
    allow = ~np.isneginf(np.asarray(mask))
    allow_ref = ~np.triu(np.ones((T_, T_), bool), k=1)
    return np.array_equal(allow, allow_ref)


def make_mask_tiles(mask, causal):
    """per-core multiplicative mask tiles (bf16 0/1), key-order swapped for odd cores.

    Merged q-chunk pairs: positions {2mp, 2mp+1} share one N=128 block.
    Causal: one [128, 2, 128] tile — [:,0,:] masks key tile kt=2mp
    ([diag | ones]), [:,1,:] masks kt=2mp+1 ([zeros | diag]); the pattern
    is mp-independent. General: [4, 8, 128, 128] per (mp, kt)."""
    allow = ~np.isneginf(np.asarray(mask))  # [q, k] True = allowed
    tiles = []
    for core in range(8):
        par = core % 2

        def ktile_order(kt):
            k = np.arange(128 * kt, 128 * kt + 128)
            if par == 1:
                k = k.reshape(2, 64)[::-1].reshape(128)
            return k

        def qcols(mp):
            # merged block columns = positions 2mp, 2mp+1 -> chunks j=4mp+par, 4mp+2+par
            j0, j1 = 2 * (2 * mp) + par, 2 * (2 * mp + 1) + par
            return np.concatenate([np.arange(64 * j0, 64 * j0 + 64),
                                   np.arange(64 * j1, 64 * j1 + 64)])

        if causal:
            mp = 0
            m = np.zeros((128, 2, 128), dtype=(ml_dtypes.float8_e4m3 if FP8_AV else ml_dtypes.bfloat16))
            q = qcols(mp)
            for i, kt in enumerate((2 * mp, 2 * mp + 1)):
                m[:, i, :] = allow[np.ix_(q, ktile_order(kt))].T
            tiles.append(np.ascontiguousarray(m))
        else:
            m = np.zeros((4, NTT, 128, 128), dtype=(ml_dtypes.float8_e4m3 if FP8_AV else ml_dtypes.bfloat16))
            for mp in range(4):
                q = qcols(mp)
                for kt in range(NTT):
                    m[mp, kt] = allow[np.ix_(q, ktile_order(kt))].T
            tiles.append(m)
    return tiles


def prep_in_maps(inputs):
    bfl = ml_dtypes.bfloat16
    x = np.asarray(inputs['input'], np.float32)      # [B, T, C]
    mask = np.asarray(inputs['mask'], np.float32)
    causal = check_causal(mask)
    Wq = np.asarray(inputs['Wq'], np.float32)        # [H, C, D]
    Wk = np.asarray(inputs['Wk'], np.float32)
    Wv = np.asarray(inputs['Wv'], np.float32)
    Wo = np.asarray(inputs['Wo'], np.float32)        # [C, C]
    w1 = np.asarray(inputs['w1'], np.float32)        # [C, DFF]
    w2 = np.asarray(inputs['w2'], np.float32)
    w3 = np.asarray(inputs['w3'], np.float32)        # [DFF, C]

    f8l = ml_dtypes.float8_e4m3
    qk_dt = f8l if FP8_QK else bfl
    qk_s = S_QK if FP8_QK else 1.0

    def pmaj(a, nct):
        """[X, (ct p), d] -> partition-major [X, p, ct, d] (contiguous per-p
        DMA lines)"""
        x, cpd, dd = a.shape
        return np.ascontiguousarray(
            a.reshape(x, nct, 128, dd).transpose(0, 2, 1, 3))

    wq_l = pmaj((Wq * qk_s).reshape(NP, 2, C, D).transpose(0, 2, 1, 3)
                .reshape(NP, C, 128), NCT).astype(qk_dt)
    wk_l = pmaj((Wk * qk_s).reshape(NP, 2, C, D).transpose(0, 2, 1, 3)
                .reshape(NP, C, 128), NCT).astype(qk_dt)
    wv_l = pmaj((Wv * (S_V if FP8_V else 1.0)).reshape(4, 4, C, D)
                .transpose(0, 2, 1, 3).reshape(4, C, 256),
                NCT).astype(f8l if FP8_V else bfl)
    # wo: [NCT, (half ci p), d] -> [NCT, half, p, ci, d]
    wo_l = np.ascontiguousarray(
        Wo.reshape(C, NCT, 128).transpose(1, 0, 2)
        .reshape(NCT, 2, 4, 128, 128).transpose(0, 1, 3, 2, 4)).astype(bfl)
    w1s = (w1 * S1).reshape(C, 32, 128).transpose(1, 0, 2)  # [32, C, 128]
    if FP8_GATE_FULL:
        w1f_l = pmaj(w1s, NCT).astype(f8l)
        w1a_l = w1b_l = None
    else:
        w1a_l = pmaj(np.ascontiguousarray(w1s[:, :C // 2]),
                     4).astype(f8l if FP8_GATE4 else bfl)
        w1b_l = pmaj(np.ascontiguousarray(w1s[:, C // 2:]), 4).astype(bfl)
    w2_l = pmaj((w2 * (S2 if FP8_VAL else 1.0)).reshape(C, 32, 128)
                .transpose(1, 0, 2), NCT).astype(f8l if FP8_VAL else bfl)
    # w3: [NCT, (dc ft p), d] -> [NCT, dc, p, ft, d]
    w3_l = np.ascontiguousarray(
        (w3 * (S3 if FP8_GVW3 else 1.0)).reshape(DFF, NCT, 128)
        .transpose(1, 0, 2).reshape(NCT, 4, 8, 128, 128)
        .transpose(0, 1, 3, 2, 4)).astype(f8l if FP8_GVW3 else bfl)

    def packp(v):
        return np.ascontiguousarray(np.asarray(v, np.float32).reshape(-1, 128).T)

    cvec_l = np.ascontiguousarray(np.stack(
        [packp(inputs['g1']), packp(inputs['be1']), packp(inputs['g2']),
         packp(inputs['be2']), packp(inputs['bo']), packp(inputs['b3'])],
        axis=1))
    # b2 is added to pvl, which carries the S2 weight scale
    bvec_l = np.ascontiguousarray(np.stack(
        [packp(inputs['b1']),
         packp(np.asarray(inputs['b2'], np.float32) * (S2 if FP8_VAL else 1.0))],
        axis=1))

    mask_tiles = make_mask_tiles(mask, causal)

    in_maps = []
    for core in range(8):
        b, par = core // 2, core % 2
        xt_c = np.ascontiguousarray(x[b].T)            # [C, T]
        if par == 1:
            xt_c = np.ascontiguousarray(swap_cols64(xt_c))
        im = dict(
            xt=xt_c, maskt=mask_tiles[core],
            wq=wq_l, wk=wk_l, wv=wv_l, wo=wo_l, w2=w2_l, w3=w3_l,
            cvec=cvec_l, bvec=bvec_l)
        im['ident'] = np.eye(128, dtype=bfl)
        if FP8_GATE_FULL:
            im['w1f'] = w1f_l
        else:
            im['w1a'] = w1a_l
            im['w1b'] = w1b_l
        in_maps.append(im)
    szflags = dict(
        ln1=bool(np.all(np.asarray(inputs['g1']) == 1)
                 and np.all(np.asarray(inputs['be1']) == 0)),
        ln2=bool(np.all(np.asarray(inputs['g2']) == 1)
                 and np.all(np.asarray(inputs['be2']) == 0)),
        bo=bool(np.all(np.asarray(inputs['bo']) == 0)),
        b1=bool(np.all(np.asarray(inputs['b1']) == 0)),
        b2=bool(np.all(np.asarray(inputs['b2']) == 0)),
        b3=bool(np.all(np.asarray(inputs['b3']) == 0)),
    )
    return in_maps, causal, szflags


def assemble(outs, B=4):
    """outs: list of 8 per-core dicts with 'out' [C, TOK] -> [B, T, C]"""
    full = np.zeros((B, T, C), np.float32)
    for core in range(8):
        b, par = core // 2, core % 2
        o = np.asarray(outs[core]['out']).reshape(C, NCH, 64)
        for p in range(NCH):
            j = 2 * p + par
            full[b, 64 * j:64 * j + 64, :] = o[:, p, :].T
    return full


# ===================== entry point =====================

_NC_CACHE = {}


def _get_nc(causal, sz):
    key = (causal, tuple(sorted(sz.items())))
    if key not in _NC_CACHE:
        _NC_CACHE[key] = build_nc(causal=causal, sz=sz, silu_act=True)
    return _NC_CACHE[key]


def run_on_hw(inputs):
    from concourse import bass2jax
    in_maps, causal, sz = prep_in_maps(inputs)
    nc = _get_nc(causal, sz)
    results = bass2jax.run_bass_via_pjrt(nc, in_maps, n_cores=8)
    return assemble(results)


def kernel(**inputs):
    return run_on_hw(inputs)

